# revision 51
# baseline (speedup 1.0000x reference)
"""Causal attention (B=4, S=4096, D=64, fp32) on 8 Trainium2 NeuronCores.

v3c: all gangs except the short-row chunk run fp8e4m3 DoubleRow PV —
one matmul per 2-tile gang contracting 256 keys at 2 MACs/cell/cycle.
Logits are shifted by C=2 before exp (softmax-invariant) so
unnormalized probs fit e4m3's +-240 range. The exp fans out over THREE
engines: ACT (exact exp -> fp8, bit-exact RNE), DVE and GPSIMD
(Schraudolph affine -> saturating uint8 viewed as fp8e4m3; negatives
clamp to +0). Diagonal gangs fuse the causal mask into the Schraudolph
bias tensor (scalar_tensor_tensor with -1000 on masked lanes -> byte
saturates to +0), eliminating the separate mask multiply. The
short-row chunk (C=4) keeps the exact fp16 path.

Layout as v2: scores transposed S^T[k,q] (d on partitions), QK packed
2-up on the PE via row-group tiling, normalization deferred via a
ones-column in V accumulating row sums.
"""

import numpy as np
import ml_dtypes

import jax
import concourse.bass as bass  # noqa: F401
import concourse.mybir as mybir
from concourse import bacc
from concourse import bass2jax
from concourse.tile import TileContext

B, S, D = 4, 4096, 64
NCORES = 8
SLOT_A = (12, 4, 24, 32)  # program A: chunks {2,0,5,7} of a batch (72 tiles)
SLOT_B = (8, 16, 20, 28)  # program B: chunks {1,3,4,6} (72 tiles)
F32 = mybir.dt.float32
F16 = mybir.dt.float16
F8 = mybir.dt.float8e4
U8 = mybir.dt.uint8
I16 = mybir.dt.int16
E4NP = ml_dtypes.float8_e4m3

LOG2E = 1.4426950408889634
CSH = 2.0  # logit shift: p = exp(score/8 - CSH), softmax-invariant
# fp16 Schraudolph (int16 bit trick), shift folded into the bias
SCH_A = 0.125 * 1024 * LOG2E
SCH_B = (15.0 - 0.0435) * 1024.0 - CSH * 1024.0 * LOG2E
# fp8 Schraudolph (saturating uint8 viewed as e4m3), shift in bias
SCH_A8 = 0.125 * 8.0 * LOG2E
SCH_B8 = 56.0 - CSH * 8.0 * LOG2E - 0.40
MASKB = -1000.0

_cache = {}


def _chunk_index(slot_c, m):
    return slot_c[m] // 4 - 1


def _build_program(slot_c, warmup_n, act_seed=900.0):
    n_shared = [max(c - 8, 0) for c in slot_c]
    n_slab = [min(c, 8) for c in slot_c]
    nb_shared = [ns // 2 for ns in n_shared]
    n_pr = [c // 2 for c in slot_c]  # V pairs per chunk (all tiles)
    max_nb = max(nb_shared)
    max_nb8 = max(n_pr)
    W8 = 80 * max_nb8

    nc = bacc.Bacc("TRN2", target_bir_lowering=False, debug=False)
    qt_d = nc.declare_dram_parameter("qt", [128, 2048], F16, isOutput=False)
    ktm_d = nc.declare_dram_parameter(
        "ktm", [128, 128 * max(max_nb, 1)], F16, isOutput=False
    )
    kts_d = nc.declare_dram_parameter("kts", [128, 2048], F16, isOutput=False)
    vm8_d = nc.declare_dram_parameter("vm8", [128, 2, W8], F8, isOutput=False)
    vs_d = nc.declare_dram_parameter("vs", [128, 260], F16, isOutput=False)
    mk_d = nc.declare_dram_parameter("mask", [128, 512], F16, isOutput=False)
    mb_d = nc.declare_dram_parameter("mb", [128, 2, 1024], F16, isOutput=False)
    o_d = nc.declare_dram_parameter("o", [65, 2048], F32, isOutput=True)
    EXP = mybir.ActivationFunctionType.Exp
    MUL = mybir.AluOpType.mult
    ADD = mybir.AluOpType.add

    # ---- exp engine plan (greedy balance, built at trace time) ----
    act_t, dve_t, gp_t = [act_seed], [0.0], [0.0]

    def plan_exp(cols, allow=("act", "dve")):
        costs = {
            "act": 0.87 * cols + 90.0,
            "dve": 1.0417 * cols + 70.0,
            "gp": 1.39 * cols + 265.0,
        }
        tot = {"act": act_t[0], "dve": dve_t[0], "gp": gp_t[0]}
        eng = min(allow, key=lambda e: tot[e] + costs[e])
        if eng == "act":
            act_t[0] += costs["act"]
        elif eng == "dve":
            dve_t[0] += costs["dve"]
        else:
            gp_t[0] += costs["gp"]
        return eng

    with TileContext(nc) as tc:
        with (
            tc.tile_pool(name="cons", bufs=1) as cons,
            tc.tile_pool(name="data", bufs=1) as data,
            tc.tile_pool(name="pp", bufs=5) as pp,
            tc.tile_pool(name="ep", bufs=2) as ep,
            tc.tile_pool(name="ps_sc", bufs=3, space="PSUM") as ps_sc,
            tc.tile_pool(name="ps_acc", bufs=2, space="PSUM") as ps_acc,
        ):
            warm = cons.tile([128, 512], F16)
            biasC = cons.tile([128, 1], F32)
            dummy = cons.tile([128, 1], F32)
            nc.vector.memset(warm[:, 0:256], 0.0)
            nc.gpsimd.memset(warm[:, 256:512], 0.0)
            nc.vector.memset(biasC[:], -CSH)
            # pull ACT_TABLE_LOAD to the front of the Scalar queue so the
            # first real exp isn't serialized behind a 1.3us table load
            nc.scalar.activation(
                dummy[:], biasC[:], EXP, scale=1.0, bias=biasC[:]
            )
            for w in range(warmup_n):
                # alternate subtiles so every sc byte is finite before any
                # full-width diag read (uninit PSUM can be NaN)
                wp = ps_sc.tile([128, 2, 512], F32, tag="sc")
                nc.tensor.matmul(
                    wp[:, w % 2, :], warm[:, 0:128], warm[:], start=True, stop=True
                )

            qt = data.tile([128, 2048], F16)
            kts = data.tile([128, 2048], F16)
            vs = data.tile([128, 260], F16)
            ktm = data.tile([128, 128 * max(max_nb, 1)], F16)
            vm8 = data.tile([128, 2, W8], F8)
            mb = cons.tile([128, 2, 1024], F16)  # [:, :, 0:512]=mb01, 512:1024=mb23

            def dma_slot(m):
                nsb = n_slab[m] // 2
                nc.sync.dma_start(
                    out=kts[:, 512 * m : 512 * m + 128 * nsb],
                    in_=kts_d[:, 512 * m : 512 * m + 128 * nsb],
                )
                if slot_c[m] == 4:
                    nc.sync.dma_start(out=vs[:], in_=vs_d[:])

            def dma_k(lo, hi):  # shared k blocks (pairs) [lo, hi)
                if hi <= lo:
                    return
                nc.sync.dma_start(
                    out=ktm[:, 128 * lo : 128 * hi], in_=ktm_d[:, 128 * lo : 128 * hi]
                )

            def dma_v8(lo, hi):  # V pairs [lo, hi)
                if hi <= lo:
                    return
                nc.sync.dma_start(
                    out=vm8[:, :, 80 * lo : 80 * hi], in_=vm8_d[:, :, 80 * lo : 80 * hi]
                )

            mask = cons.tile([128, 512], F16)
            done_k = 0
            done_v = 0
            nc.sync.dma_start(out=qt[:, 0:512], in_=qt_d[:, 0:512])
            for m in range(4):
                # this chunk's K data first, then next chunk's qt, then V
                while done_k < nb_shared[m]:
                    step = min(3, nb_shared[m] - done_k)
                    dma_k(done_k, done_k + step)
                    done_k += step
                dma_slot(m)
                if m < 3:
                    nc.sync.dma_start(
                        out=qt[:, 512 * (m + 1) : 512 * (m + 2)],
                        in_=qt_d[:, 512 * (m + 1) : 512 * (m + 2)],
                    )
                while done_v < n_pr[m]:
                    step = min(4, n_pr[m] - done_v)
                    dma_v8(done_v, done_v + step)
                    done_v += step
                if m == 0:
                    nc.sync.dma_start(out=mask[:], in_=mk_d[:])
                    nc.sync.dma_start(out=mb[:], in_=mb_d[:])
                fut_k = max(nb_shared[m:])
                if done_k < fut_k:
                    step = min(3, fut_k - done_k)
                    dma_k(done_k, done_k + step)
                    done_k += step
                fut_v = max(n_pr[m:])
                if done_v < fut_v:
                    step = min(4, fut_v - done_v)
                    dma_v8(done_v, done_v + step)
                    done_v += step

            pending = []  # (emit_fn, pt, gang, after_fn) across chunks
            gang_no = [0]  # global gang counter (plug placement)
            dr_no = [0]  # global DR-gang counter (fill split)

            def pump(limit):
                while len(pending) > limit:
                    fn, pt_, gang_, after = pending.pop(0)
                    fn(pt_, gang_)
                    if after is not None:
                        after()

            for m in range(4):
                C = slot_c[m]
                ns = n_shared[m]
                diag_first = m == 3 and ns >= 4
                short_rows = C == 4  # chunk 0: exact fp16 path
                q_sl = slice(512 * m, 512 * (m + 1))
                acc = ps_acc.tile([65, 512], F32, tag="acc")

                def tile_geom(t, C=C, ns=ns, diag_first=diag_first):
                    g = (t - ns) if diag_first else (t - (C - 4))
                    if 0 <= g <= 3:
                        off = 128 * g
                    else:
                        off = 0
                    return (g if 0 <= g <= 3 else -1), off, 512 - off

                def dr_pair(t0, C=C, ns=ns, diag_first=diag_first):
                    # vm8 pair index for the gang starting at position t0
                    if diag_first:
                        if ns <= t0 < ns + 4:
                            return (C - 4 + (t0 - ns)) // 2
                        if t0 >= ns + 4:
                            return (t0 - 4) // 2
                    return t0 // 2

                def emit_pv(
                    pt, gang, C=C, m=m, acc=acc,
                    tile_geom=tile_geom, dr_pair=dr_pair, short_rows=short_rows,
                ):
                    t0 = gang[0]
                    if not short_rows:
                        # fp8 DoubleRow matmul for both tiles (diag incl.)
                        pr = dr_pair(t0)
                        pt8 = pt.bitcast(F8)
                        nc.tensor.matmul(
                            acc[:],
                            vm8[:, :, 80 * pr : 80 * pr + 65],
                            pt8[:, :, 0:512],
                            start=(t0 == 0),
                            stop=(gang[1] == C - 1),
                            perf_mode=mybir.MatmulPerfMode.DoubleRow,
                        )
                        return
                    for j, t in enumerate(gang):
                        g, off, w = tile_geom(t)
                        ptile = pt[:, j, 0:w]
                        nc.vector.tensor_mul(ptile, ptile, mask[:, :w])
                        vt = vs[:, 65 * g : 65 * (g + 1)]
                        nc.tensor.matmul(
                            acc[:, off:512],
                            vt,
                            ptile,
                            start=(t == 0),
                            stop=(t == C - 1),
                        )

                def make_epilogue(m=m, acc=acc, last=(m == 3)):
                    def epilogue():
                        osb = ep.tile([65, 512], F32, tag="osb")
                        if last:
                            # final chunk: copy+DMA in two halves on DVE so
                            # the first half's DMA overlaps the second copy
                            nc.vector.tensor_copy(osb[:, 0:256], acc[:, 0:256])
                            nc.sync.dma_start(
                                out=o_d[:, 512 * m : 512 * m + 256],
                                in_=osb[:, 0:256],
                            )
                            nc.vector.tensor_copy(
                                osb[:, 256:512], acc[:, 256:512]
                            )
                            dve_t[0] += 800.0
                        else:
                            # split the PSUM->SBUF copy across DVE and ACT
                            nc.vector.tensor_copy(osb[:, 0:256], acc[:, 0:256])
                            nc.scalar.activation(
                                osb[:, 256:512],
                                acc[:, 256:512],
                                mybir.ActivationFunctionType.Copy,
                            )
                            act_t[0] += 360.0
                            dve_t[0] += 360.0
                        if last:
                            nc.sync.dma_start(
                                out=o_d[:, 512 * m + 256 : 512 * (m + 1)],
                                in_=osb[:, 256:512],
                            )
                        else:
                            nc.sync.dma_start(
                                out=o_d[:, 512 * m : 512 * (m + 1)], in_=osb[:]
                            )

                    return epilogue

                n_gangs = (C + 1) // 2
                positions = list(range(0, C, 2))
                if not short_rows and C >= 12:
                    # spread the two diag gangs (forced-DVE stt) between DR
                    # gangs so ACT isn't starved at chunk tails
                    if diag_first:
                        i = positions.index(ns)
                        positions[i + 1], positions[i + 2] = (
                            positions[i + 2], positions[i + 1],
                        )
                    else:
                        positions[-3], positions[-2] = (
                            positions[-2], positions[-3],
                        )
                for gi, t0 in enumerate(positions):
                    gang = (t0, t0 + 1)
                    sc = ps_sc.tile([128, 2, 512], F32, tag="sc")
                    geoms = []
                    for j, t in enumerate(gang):
                        g, off, w = tile_geom(t)
                        geoms.append((g, off, w))
                        if t < ns:
                            blk = t // 2
                            lhsT = ktm[
                                64 * j : 64 * (j + 1), 128 * blk : 128 * (blk + 1)
                            ]
                        else:
                            p = t - ns
                            blk = p // 2
                            lhsT = kts[
                                64 * j : 64 * (j + 1),
                                512 * m + 128 * blk : 512 * m + 128 * (blk + 1),
                            ]
                        rhs = qt[64 * j : 64 * (j + 1), q_sl]
                        if off:
                            rhs = rhs[:, off:512]
                        # diag scores land at their true q-columns so the
                        # full-width DoubleRow PV accumulates them unshifted;
                        # the short-row fp16 path keeps the 0-based window
                        dst = (
                            sc[:, j, 0:w] if short_rows else sc[:, j, off:512]
                        )
                        nc.tensor.matmul(
                            dst,
                            lhsT,
                            rhs,
                            start=True,
                            stop=True,
                        )
                    pt = pp.tile([128, 2, 512], F16, tag="pt")
                    is_diag = any(g >= 0 for g, _, _ in geoms)
                    if short_rows:
                        # fp16 path, mask applied via tensor_mul later.
                        # Even tiles exact on ACT (covers the shortest rows);
                        # odd tiles Schraudolph-fp16 on DVE (rows there have
                        # >=129 keys, enough wash-out).
                        pti = pt.bitcast(I16)
                        for j, (g, off, w) in enumerate(geoms):
                            if gang[j] % 2 == 0:
                                nc.scalar.activation(
                                    pt[:, j, 0:w], sc[:, j, 0:w], EXP,
                                    scale=0.125, bias=biasC[:],
                                )
                                act_t[0] += 0.87 * w + 90.0
                            else:
                                nc.vector.tensor_scalar(
                                    pti[:, j, 0:w], sc[:, j, 0:w],
                                    SCH_A, SCH_B, MUL, ADD,
                                )
                                dve_t[0] += 1.0417 * w + 70.0
                        dve_t[0] += 0.52 * sum(w for _, _, w in geoms) + 300
                    elif is_diag:
                        # fused exp+mask Schraudolph -> fp8 on DVE, narrowed
                        # to the valid window; GPSIMD zeroes the stale cols
                        ptu8 = pt.bitcast(U8)
                        v = 0 if geoms[0][0] == 0 else 1  # mb01 or mb23
                        for j, (g, off, w) in enumerate(geoms):
                            if off:
                                nc.gpsimd.memset(ptu8[:, j, 0:off], 0)
                            mbs = mb[:, j, 512 * v + off : 512 * (v + 1)]
                            plan_exp(w, allow=("dve",))
                            nc.vector.scalar_tensor_tensor(
                                ptu8[:, j, off:512], sc[:, j, off:512],
                                SCH_A8, mbs, MUL, ADD,
                            )
                    else:
                        pt8 = pt.bitcast(F8)
                        ptu8 = pt.bitcast(U8)
                        if dr_no[0] < 3:
                            # pipeline fill: split first gangs across engines
                            nc.scalar.activation(
                                pt8[:, 0, 0:512], sc[:, 0, :], EXP,
                                scale=0.125, bias=biasC[:],
                            )
                            nc.vector.tensor_scalar(
                                ptu8[:, 1, 0:512], sc[:, 1, :],
                                SCH_A8, SCH_B8, MUL, ADD,
                            )
                            act_t[0] += 0.87 * 512 + 90.0
                            dve_t[0] += 1.0417 * 512 + 70.0
                        else:
                            # tail: keep DVE free for the final copy chain
                            force = (
                                ("act",)
                                if (m == 3 and gi >= n_gangs - 2)
                                else ("act", "dve")
                            )
                            eng = plan_exp(1024, allow=force)
                            if eng == "act":
                                nc.scalar.activation(
                                    pt8[:, :, 0:512], sc[:], EXP,
                                    scale=0.125, bias=biasC[:],
                                )
                            else:
                                nc.vector.tensor_scalar(
                                    ptu8[:, :, 0:512], sc[:],
                                    SCH_A8, SCH_B8, MUL, ADD,
                                )
                    after = make_epilogue() if gi == n_gangs - 1 else None
                    pending.append((emit_pv, pt, gang, after))
                    if not is_diag and not short_rows:
                        dr_no[0] += 1
                    gang_no[0] += 1
                    # plugs must stop before the first start=False PV pop
                    # (they'd clobber live accumulation in the acc bufs)
                    if gang_no[0] <= 3:
                        # fill plugs: dependency-free PE work in the acc bank
                        # keeps the PE dense (HAM-warm) while the first
                        # gangs' exp results are still in flight (the next
                        # real PV's start=True clears the plug garbage)
                        for _ in range(2 + (gang_no[0] <= 2)):
                            wpa = ps_acc.tile([65, 512], F32, tag="acc")
                            nc.tensor.matmul(
                                wpa[:], warm[:, 0:65], warm[:],
                                start=True, stop=True,
                            )
                    pump(2 if m == 0 else 3)
            pump(0)

    nc.compile()
    return nc


def _prep_core_inputs(slot_c, b, query, key, value):
    n_shared = [max(c - 8, 0) for c in slot_c]
    n_slab = [min(c, 8) for c in slot_c]
    nb_shared = [ns // 2 for ns in n_shared]
    n_pr = [c // 2 for c in slot_c]
    max_nb = max(nb_shared)
    max_nb8 = max(n_pr)
    W8 = 80 * max_nb8

    qt = np.zeros((128, 2048), np.float16)
    kts = np.zeros((128, 2048), np.float16)
    vs = np.zeros((128, 260), np.float16)
    # ktm: block j holds tiles 2j (rows 0-63) and 2j+1 (rows 64-127)
    ktm = np.zeros((128, 128 * max(max_nb, 1)), np.float16)
    kT = key[b].T.astype(np.float16)  # [64, S]
    for j in range(max_nb):
        ktm[0:64, 128 * j : 128 * (j + 1)] = kT[:, 128 * (2 * j) : 128 * (2 * j + 1)]
        ktm[64:128, 128 * j : 128 * (j + 1)] = kT[
            :, 128 * (2 * j + 1) : 128 * (2 * j + 2)
        ]
    vaug = np.ones((S, 65), np.float16)
    vaug[:, :64] = value[b]
    # vm8: V pairs (all tiles), fp8e4m3, padded to stride 80
    vm8 = np.zeros((128, 2, W8), E4NP)
    v8 = vaug.astype(E4NP)  # RNE quantization
    for p in range(max_nb8):
        for j in range(2):
            t = 2 * p + j
            vm8[:, j, 80 * p : 80 * p + 65] = v8[128 * t : 128 * (t + 1), :]
    for m in range(4):
        c = _chunk_index(slot_c, m)
        n = slot_c[m]
        diag_first = m == 3 and n_shared[m] >= 4
        qchunk = query[b, 512 * c : 512 * (c + 1), :].T.astype(np.float16)
        qt[0:64, 512 * m : 512 * (m + 1)] = qchunk
        qt[64:128, 512 * m : 512 * (m + 1)] = qchunk
        for p in range(n_slab[m]):
            if diag_first:
                t = (n - 4 + p) if p < 4 else (n - 8 + (p - 4))
            else:
                t = n_shared[m] + p
            row = slice(0, 64) if p % 2 == 0 else slice(64, 128)
            col = slice(512 * m + 128 * (p // 2), 512 * m + 128 * (p // 2 + 1))
            kts[row, col] = key[b, 128 * t : 128 * (t + 1), :].T
        if n == 4:  # short-row chunk: fp16 V for tiles 0..3
            for g in range(4):
                vs[:, 65 * g : 65 * (g + 1)] = vaug[128 * g : 128 * (g + 1), :]
    mask = np.triu(np.ones((128, 512), dtype=np.float16))
    # mb: fused Schraudolph-bias masks for diag gangs, [128, 2, 1024]
    # variant v=0 (tiles g=0,1), v=1 (tiles g=2,3); scores sit at their true
    # q-columns, so cols < off are stale -> MASKB, the [off, off+128) block
    # is the causal triangle, and cols >= off+128 are fully allowed
    mb = np.full((128, 2, 1024), MASKB, np.float16)
    tri128 = np.triu(np.ones((128, 128), bool))
    for v in range(2):
        for j in range(2):
            off = 128 * (2 * v + j)
            blk = np.full((128, 512), MASKB, np.float16)
            blk[:, off : off + 128] = np.where(
                tri128, np.float16(SCH_B8), np.float16(MASKB)
            )
            blk[:, off + 128 :] = np.float16(SCH_B8)
            mb[:, j, 512 * v : 512 * (v + 1)] = blk
    return {
        "qt": qt, "ktm": ktm, "kts": kts, "vm8": vm8, "vs": vs,
        "mask": mask, "mb": mb,
    }


def _make_runner(nc, devices):
    """Vendored multi-core run_bass_via_pjrt with an explicit device set,
    split into an async dispatch and a blocking unpack."""
    from jax.sharding import Mesh, PartitionSpec

    bass2jax.install_neuronx_cc_hook()
    n = len(devices)
    partition_name = nc.partition_id_tensor.name if nc.partition_id_tensor else None
    in_names, out_names, out_avals, zero_outs = [], [], [], []
    for alloc in nc.m.functions[0].allocations:
        if not isinstance(alloc, mybir.MemoryLocationSet):
            continue
        name = alloc.memorylocations[0].name
        if alloc.kind == "ExternalInput":
            if name != partition_name:
                in_names.append(name)
        elif alloc.kind == "ExternalOutput":
            out_names.append(name)
            shape = tuple(alloc.tensor_shape)
            dtype = mybir.dt.np(alloc.dtype)
            out_avals.append(jax.core.ShapedArray(shape, dtype))
            zero_outs.append(np.zeros(shape, dtype))
    n_params = len(in_names)
    all_in = list(in_names) + list(out_names)
    if partition_name is not None:
        all_in.append(partition_name)
    all_in = tuple(all_in)
    donate = tuple(range(n_params, n_params + len(out_names)))

    def _body(*args):
        operands = list(args)
        if partition_name is not None:
            operands.append(bass2jax.partition_id_tensor())
        outs = bass2jax._bass_exec_p.bind(
            *operands,
            out_avals=tuple(out_avals),
            in_names=all_in,
            out_names=tuple(out_names),
            lowering_input_output_aliases=(),
            sim_require_finite=True,
            sim_require_nnan=True,
            nc=nc,
        )
        return tuple(outs)

    mesh = Mesh(np.asarray(devices), ("core",))
    in_specs = (PartitionSpec("core"),) * (n_params + len(out_names))
    out_specs = (PartitionSpec("core"),) * len(out_names)
    sharded = jax.jit(
        jax.shard_map(
            _body, mesh=mesh, in_specs=in_specs, out_specs=out_specs, check_vma=False
        ),
        donate_argnums=donate,
        keep_unused=True,
    )

    def dispatch(in_maps):
        concat_in = [
            np.concatenate([np.asarray(in_maps[c][nm]) for c in range(n)], axis=0)
            for nm in in_names
        ]
        concat_zeros = [
            np.zeros((n * z.shape[0], *z.shape[1:]), z.dtype) for z in zero_outs
        ]
        return sharded(*concat_in, *concat_zeros)

    def unpack(out_arrs):
        return [
            {
                nm: np.asarray(out_arrs[i]).reshape(n, *out_avals[i].shape)[c]
                for i, nm in enumerate(out_names)
            }
            for c in range(n)
        ]

    return dispatch, unpack


def _get_engine():
    if "engine" not in _cache:
        devs = jax.devices()
        ncA = _build_program(SLOT_A, 10, act_seed=0.0)
        ncB = _build_program(SLOT_B, 7, act_seed=0.0)
        dispA, unpackA = _make_runner(ncA, devs[0:4])
        dispB, unpackB = _make_runner(ncB, devs[4:8])
        _cache["engine"] = (dispA, unpackA, dispB, unpackB)
        _cache["ncs"] = (ncA, ncB)
    return _cache["engine"]


def run(query, key, value):
    dispA, unpackA, dispB, unpackB = _get_engine()
    mapsA = [_prep_core_inputs(SLOT_A, b, query, key, value) for b in range(4)]
    mapsB = [_prep_core_inputs(SLOT_B, b, query, key, value) for b in range(4)]
    outA = dispA(mapsA)
    outB = dispB(mapsB)
    resA = unpackA(outA)
    resB = unpackB(outB)

    out = np.zeros((B, S, D), np.float32)
    for b in range(4):
        for slot_c, res in ((SLOT_A, resA[b]), (SLOT_B, resB[b])):
            o = res["o"]  # [65, 2048]: chunk m at cols [512m, 512m+512)
            for m in range(4):
                c = _chunk_index(slot_c, m)
                blk = o[:, 512 * m : 512 * (m + 1)]
                out[b, 512 * c : 512 * (c + 1), :] = (blk[:64] / blk[64]).T
    return out


def kernel(query, key, value):
    query = np.ascontiguousarray(np.asarray(query, dtype=np.float32))
    key = np.ascontiguousarray(np.asarray(key, dtype=np.float32))
    value = np.ascontiguousarray(np.asarray(value, dtype=np.float32))
    return run(query, key, value)


# revision 52
# speedup vs baseline: 1.0348x; 1.0348x over previous
"""Causal attention (B=4, S=4096, D=64, fp32) on 8 Trainium2 NeuronCores.

v3c: all gangs except the short-row chunk run fp8e4m3 DoubleRow PV —
one matmul per 2-tile gang contracting 256 keys at 2 MACs/cell/cycle.
Logits are shifted by C=2 before exp (softmax-invariant) so
unnormalized probs fit e4m3's +-240 range. The exp fans out over THREE
engines: ACT (exact exp -> fp8, bit-exact RNE), DVE and GPSIMD
(Schraudolph affine -> saturating uint8 viewed as fp8e4m3; negatives
clamp to +0). Diagonal gangs fuse the causal mask into the Schraudolph
bias tensor (scalar_tensor_tensor with -1000 on masked lanes -> byte
saturates to +0), eliminating the separate mask multiply. The
short-row chunk (C=4) keeps the exact fp16 path.

Layout as v2: scores transposed S^T[k,q] (d on partitions), QK packed
2-up on the PE via row-group tiling, normalization deferred via a
ones-column in V accumulating row sums.
"""

import numpy as np
import ml_dtypes

import jax
import concourse.bass as bass  # noqa: F401
import concourse.mybir as mybir
from concourse import bacc
from concourse import bass2jax
from concourse.tile import TileContext

B, S, D = 4, 4096, 64
NCORES = 8
SLOT_A = (12, 4, 24, 32)  # program A: chunks {2,0,5,7} of a batch (72 tiles)
SLOT_B = (8, 16, 20, 28)  # program B: chunks {1,3,4,6} (72 tiles)
F32 = mybir.dt.float32
F16 = mybir.dt.float16
F8 = mybir.dt.float8e4
U8 = mybir.dt.uint8
I16 = mybir.dt.int16
E4NP = ml_dtypes.float8_e4m3

LOG2E = 1.4426950408889634
CSH = 2.0  # logit shift: p = exp(score/8 - CSH), softmax-invariant
# fp16 Schraudolph (int16 bit trick), shift folded into the bias
SCH_A = 0.125 * 1024 * LOG2E
SCH_B = (15.0 - 0.0435) * 1024.0 - CSH * 1024.0 * LOG2E
# fp8 Schraudolph (saturating uint8 viewed as e4m3), shift in bias
SCH_A8 = 0.125 * 8.0 * LOG2E
SCH_B8 = 56.0 - CSH * 8.0 * LOG2E - 0.40
MASKB = -1000.0

_cache = {}


def _chunk_index(slot_c, m):
    return slot_c[m] // 4 - 1


def _build_program(slot_c, warmup_n, act_seed=900.0):
    n_shared = [max(c - 8, 0) for c in slot_c]
    n_slab = [min(c, 8) for c in slot_c]
    nb_shared = [ns // 2 for ns in n_shared]
    n_pr = [c // 2 for c in slot_c]  # V pairs per chunk (all tiles)
    max_nb = max(nb_shared)
    max_nb8 = max(n_pr)
    W8 = 80 * max_nb8

    nc = bacc.Bacc("TRN2", target_bir_lowering=False, debug=False)
    qt_d = nc.declare_dram_parameter("qt", [128, 2048], F16, isOutput=False)
    ktm_d = nc.declare_dram_parameter(
        "ktm", [128, 128 * max(max_nb, 1)], F16, isOutput=False
    )
    kts_d = nc.declare_dram_parameter("kts", [128, 2048], F16, isOutput=False)
    vm8_d = nc.declare_dram_parameter("vm8", [128, 2, W8], F8, isOutput=False)
    vs_d = nc.declare_dram_parameter("vs", [128, 260], F16, isOutput=False)
    mk_d = nc.declare_dram_parameter("mask", [128, 512], F16, isOutput=False)
    mb_d = nc.declare_dram_parameter("mb", [128, 2, 1024], F16, isOutput=False)
    o_d = nc.declare_dram_parameter("o", [65, 2048], F32, isOutput=True)
    EXP = mybir.ActivationFunctionType.Exp
    MUL = mybir.AluOpType.mult
    ADD = mybir.AluOpType.add

    # ---- exp engine plan (greedy balance, built at trace time) ----
    act_t, dve_t, gp_t = [act_seed], [0.0], [0.0]

    def plan_exp(cols, allow=("act", "dve")):
        costs = {
            "act": 0.87 * cols + 90.0,
            "dve": 1.0417 * cols + 70.0,
            "gp": 1.39 * cols + 265.0,
        }
        tot = {"act": act_t[0], "dve": dve_t[0], "gp": gp_t[0]}
        eng = min(allow, key=lambda e: tot[e] + costs[e])
        if eng == "act":
            act_t[0] += costs["act"]
        elif eng == "dve":
            dve_t[0] += costs["dve"]
        else:
            gp_t[0] += costs["gp"]
        return eng

    with TileContext(nc) as tc:
        with (
            tc.tile_pool(name="cons", bufs=1) as cons,
            tc.tile_pool(name="data", bufs=1) as data,
            tc.tile_pool(name="pp", bufs=5) as pp,
            tc.tile_pool(name="ep", bufs=2) as ep,
            tc.tile_pool(name="ps_sc", bufs=3, space="PSUM") as ps_sc,
            tc.tile_pool(name="ps_acc", bufs=2, space="PSUM") as ps_acc,
        ):
            warm = cons.tile([128, 512], F16)
            biasC = cons.tile([128, 1], F32)
            dummy = cons.tile([128, 1], F32)
            nc.vector.memset(warm[:, 0:256], 0.0)
            nc.gpsimd.memset(warm[:, 256:512], 0.0)
            nc.vector.memset(biasC[:], -CSH)
            # pull ACT_TABLE_LOAD to the front of the Scalar queue so the
            # first real exp isn't serialized behind a 1.3us table load
            nc.scalar.activation(
                dummy[:], biasC[:], EXP, scale=1.0, bias=biasC[:]
            )
            for w in range(warmup_n):
                # alternate subtiles so every sc byte is finite before any
                # full-width diag read (uninit PSUM can be NaN)
                wp = ps_sc.tile([128, 2, 512], F32, tag="sc")
                nc.tensor.matmul(
                    wp[:, w % 2, :], warm[:, 0:128], warm[:], start=True, stop=True
                )

            qt = data.tile([128, 2048], F16)
            kts = data.tile([128, 2048], F16)
            vs = data.tile([128, 260], F16)
            ktm = data.tile([128, 128 * max(max_nb, 1)], F16)
            vm8 = data.tile([128, 2, W8], F8)
            mb = cons.tile([128, 2, 1024], F16)  # [:, :, 0:512]=mb01, 512:1024=mb23

            def dma_slot(m):
                nsb = n_slab[m] // 2
                nc.sync.dma_start(
                    out=kts[:, 512 * m : 512 * m + 128 * nsb],
                    in_=kts_d[:, 512 * m : 512 * m + 128 * nsb],
                )
                if slot_c[m] == 4:
                    nc.sync.dma_start(out=vs[:], in_=vs_d[:])

            def dma_k(lo, hi):  # shared k blocks (pairs) [lo, hi)
                if hi <= lo:
                    return
                nc.sync.dma_start(
                    out=ktm[:, 128 * lo : 128 * hi], in_=ktm_d[:, 128 * lo : 128 * hi]
                )

            def dma_v8(lo, hi):  # V pairs [lo, hi)
                if hi <= lo:
                    return
                nc.sync.dma_start(
                    out=vm8[:, :, 80 * lo : 80 * hi], in_=vm8_d[:, :, 80 * lo : 80 * hi]
                )

            nc.sync.dma_start(out=qt[:], in_=qt_d[:])
            mask = cons.tile([128, 512], F16)
            done_k = 0
            done_v = 0
            for m in range(4):
                while done_k < nb_shared[m]:
                    step = min(3, nb_shared[m] - done_k)
                    dma_k(done_k, done_k + step)
                    done_k += step
                while done_v < n_pr[m]:
                    step = min(4, n_pr[m] - done_v)
                    dma_v8(done_v, done_v + step)
                    done_v += step
                dma_slot(m)
                if m == 0:
                    nc.sync.dma_start(out=mask[:], in_=mk_d[:])
                    nc.sync.dma_start(out=mb[:], in_=mb_d[:])
                fut_k = max(nb_shared[m:])
                if done_k < fut_k:
                    step = min(3, fut_k - done_k)
                    dma_k(done_k, done_k + step)
                    done_k += step
                fut_v = max(n_pr[m:])
                if done_v < fut_v:
                    step = min(4, fut_v - done_v)
                    dma_v8(done_v, done_v + step)
                    done_v += step

            pending = []  # (emit_fn, pt, gang, after_fn) across chunks
            gang_no = [0]  # global gang counter (plug placement)
            dr_no = [0]  # global DR-gang counter (fill split)

            def pump(limit):
                while len(pending) > limit:
                    fn, pt_, gang_, after = pending.pop(0)
                    fn(pt_, gang_)
                    if after is not None:
                        after()

            for m in range(4):
                C = slot_c[m]
                ns = n_shared[m]
                diag_first = m == 3 and ns >= 4
                short_rows = C == 4  # chunk 0: exact fp16 path
                q_sl = slice(512 * m, 512 * (m + 1))
                acc = ps_acc.tile([65, 512], F32, tag="acc")

                def tile_geom(t, C=C, ns=ns, diag_first=diag_first):
                    g = (t - ns) if diag_first else (t - (C - 4))
                    if 0 <= g <= 3:
                        off = 128 * g
                    else:
                        off = 0
                    return (g if 0 <= g <= 3 else -1), off, 512 - off

                def dr_pair(t0, C=C, ns=ns, diag_first=diag_first):
                    # vm8 pair index for the gang starting at position t0
                    if diag_first:
                        if ns <= t0 < ns + 4:
                            return (C - 4 + (t0 - ns)) // 2
                        if t0 >= ns + 4:
                            return (t0 - 4) // 2
                    return t0 // 2

                def emit_pv(
                    pt, gang, C=C, m=m, acc=acc,
                    tile_geom=tile_geom, dr_pair=dr_pair, short_rows=short_rows,
                ):
                    t0 = gang[0]
                    if not short_rows:
                        # fp8 DoubleRow matmul for both tiles (diag incl.)
                        pr = dr_pair(t0)
                        pt8 = pt.bitcast(F8)
                        nc.tensor.matmul(
                            acc[:],
                            vm8[:, :, 80 * pr : 80 * pr + 65],
                            pt8[:, :, 0:512],
                            start=(t0 == 0),
                            stop=(gang[1] == C - 1),
                            perf_mode=mybir.MatmulPerfMode.DoubleRow,
                        )
                        return
                    for j, t in enumerate(gang):
                        g, off, w = tile_geom(t)
                        ptile = pt[:, j, 0:w]
                        nc.vector.tensor_mul(ptile, ptile, mask[:, :w])
                        vt = vs[:, 65 * g : 65 * (g + 1)]
                        nc.tensor.matmul(
                            acc[:, off:512],
                            vt,
                            ptile,
                            start=(t == 0),
                            stop=(t == C - 1),
                        )

                def make_epilogue(m=m, acc=acc, last=(m == 3)):
                    def epilogue():
                        osb = ep.tile([65, 512], F32, tag="osb")
                        if last:
                            # final chunk: copy+DMA in two halves on DVE so
                            # the first half's DMA overlaps the second copy
                            nc.vector.tensor_copy(osb[:, 0:256], acc[:, 0:256])
                            nc.sync.dma_start(
                                out=o_d[:, 512 * m : 512 * m + 256],
                                in_=osb[:, 0:256],
                            )
                            nc.vector.tensor_copy(
                                osb[:, 256:512], acc[:, 256:512]
                            )
                            dve_t[0] += 800.0
                        else:
                            # split the PSUM->SBUF copy across DVE and ACT
                            nc.vector.tensor_copy(osb[:, 0:256], acc[:, 0:256])
                            nc.scalar.activation(
                                osb[:, 256:512],
                                acc[:, 256:512],
                                mybir.ActivationFunctionType.Copy,
                            )
                            act_t[0] += 360.0
                            dve_t[0] += 360.0
                        if last:
                            nc.sync.dma_start(
                                out=o_d[:, 512 * m + 256 : 512 * (m + 1)],
                                in_=osb[:, 256:512],
                            )
                        else:
                            nc.sync.dma_start(
                                out=o_d[:, 512 * m : 512 * (m + 1)], in_=osb[:]
                            )

                    return epilogue

                n_gangs = (C + 1) // 2
                positions = list(range(0, C, 2))
                if not short_rows and C >= 12:
                    # spread the two diag gangs (forced-DVE stt) between DR
                    # gangs so ACT isn't starved at chunk tails
                    if diag_first:
                        i = positions.index(ns)
                        positions[i + 1], positions[i + 2] = (
                            positions[i + 2], positions[i + 1],
                        )
                    else:
                        positions[-3], positions[-2] = (
                            positions[-2], positions[-3],
                        )
                for gi, t0 in enumerate(positions):
                    gang = (t0, t0 + 1)
                    sc = ps_sc.tile([128, 2, 512], F32, tag="sc")
                    geoms = []
                    for j, t in enumerate(gang):
                        g, off, w = tile_geom(t)
                        geoms.append((g, off, w))
                        if t < ns:
                            blk = t // 2
                            lhsT = ktm[
                                64 * j : 64 * (j + 1), 128 * blk : 128 * (blk + 1)
                            ]
                        else:
                            p = t - ns
                            blk = p // 2
                            lhsT = kts[
                                64 * j : 64 * (j + 1),
                                512 * m + 128 * blk : 512 * m + 128 * (blk + 1),
                            ]
                        rhs = qt[64 * j : 64 * (j + 1), q_sl]
                        if off:
                            rhs = rhs[:, off:512]
                        # diag scores land at their true q-columns so the
                        # full-width DoubleRow PV accumulates them unshifted;
                        # the short-row fp16 path keeps the 0-based window
                        dst = (
                            sc[:, j, 0:w] if short_rows else sc[:, j, off:512]
                        )
                        nc.tensor.matmul(
                            dst,
                            lhsT,
                            rhs,
                            start=True,
                            stop=True,
                        )
                    pt = pp.tile([128, 2, 512], F16, tag="pt")
                    is_diag = any(g >= 0 for g, _, _ in geoms)
                    if short_rows:
                        # fp16 path, mask applied via tensor_mul later.
                        # Even tiles exact on ACT (covers the shortest rows);
                        # odd tiles Schraudolph-fp16 on DVE (rows there have
                        # >=129 keys, enough wash-out).
                        pti = pt.bitcast(I16)
                        for j, (g, off, w) in enumerate(geoms):
                            if gang[j] % 2 == 0:
                                nc.scalar.activation(
                                    pt[:, j, 0:w], sc[:, j, 0:w], EXP,
                                    scale=0.125, bias=biasC[:],
                                )
                                act_t[0] += 0.87 * w + 90.0
                            else:
                                nc.vector.tensor_scalar(
                                    pti[:, j, 0:w], sc[:, j, 0:w],
                                    SCH_A, SCH_B, MUL, ADD,
                                )
                                dve_t[0] += 1.0417 * w + 70.0
                        dve_t[0] += 0.52 * sum(w for _, _, w in geoms) + 300
                    elif is_diag:
                        # fused exp+mask Schraudolph -> fp8 on DVE, narrowed
                        # to the valid window; GPSIMD zeroes the stale cols
                        ptu8 = pt.bitcast(U8)
                        v = 0 if geoms[0][0] == 0 else 1  # mb01 or mb23
                        for j, (g, off, w) in enumerate(geoms):
                            if off:
                                nc.gpsimd.memset(ptu8[:, j, 0:off], 0)
                            mbs = mb[:, j, 512 * v + off : 512 * (v + 1)]
                            plan_exp(w, allow=("dve",))
                            nc.vector.scalar_tensor_tensor(
                                ptu8[:, j, off:512], sc[:, j, off:512],
                                SCH_A8, mbs, MUL, ADD,
                            )
                    else:
                        pt8 = pt.bitcast(F8)
                        ptu8 = pt.bitcast(U8)
                        if dr_no[0] < 3:
                            # pipeline fill: split first gangs across engines
                            nc.scalar.activation(
                                pt8[:, 0, 0:512], sc[:, 0, :], EXP,
                                scale=0.125, bias=biasC[:],
                            )
                            nc.vector.tensor_scalar(
                                ptu8[:, 1, 0:512], sc[:, 1, :],
                                SCH_A8, SCH_B8, MUL, ADD,
                            )
                            act_t[0] += 0.87 * 512 + 90.0
                            dve_t[0] += 1.0417 * 512 + 70.0
                        else:
                            # tail: keep DVE free for the final copy chain
                            force = (
                                ("act",)
                                if (m == 3 and gi >= n_gangs - 2)
                                else ("act", "dve")
                            )
                            eng = plan_exp(1024, allow=force)
                            if eng == "act":
                                nc.scalar.activation(
                                    pt8[:, :, 0:512], sc[:], EXP,
                                    scale=0.125, bias=biasC[:],
                                )
                            else:
                                nc.vector.tensor_scalar(
                                    ptu8[:, :, 0:512], sc[:],
                                    SCH_A8, SCH_B8, MUL, ADD,
                                )
                    after = make_epilogue() if gi == n_gangs - 1 else None
                    pending.append((emit_pv, pt, gang, after))
                    if not is_diag and not short_rows:
                        dr_no[0] += 1
                    gang_no[0] += 1
                    # plugs must stop before the first start=False PV pop
                    # (they'd clobber live accumulation in the acc bufs)
                    if gang_no[0] <= 3:
                        # fill plugs: dependency-free PE work in the acc bank
                        # keeps the PE dense (HAM-warm) while the first
                        # gangs' exp results are still in flight (the next
                        # real PV's start=True clears the plug garbage)
                        for _ in range(2 + (gang_no[0] <= 2)):
                            wpa = ps_acc.tile([65, 512], F32, tag="acc")
                            nc.tensor.matmul(
                                wpa[:], warm[:, 0:65], warm[:],
                                start=True, stop=True,
                            )
                    pump(2 if m == 0 else 3)
            pump(0)

    nc.compile()
    return nc


def _prep_core_inputs(slot_c, b, query, key, value):
    n_shared = [max(c - 8, 0) for c in slot_c]
    n_slab = [min(c, 8) for c in slot_c]
    nb_shared = [ns // 2 for ns in n_shared]
    n_pr = [c // 2 for c in slot_c]
    max_nb = max(nb_shared)
    max_nb8 = max(n_pr)
    W8 = 80 * max_nb8

    qt = np.zeros((128, 2048), np.float16)
    kts = np.zeros((128, 2048), np.float16)
    vs = np.zeros((128, 260), np.float16)
    # ktm: block j holds tiles 2j (rows 0-63) and 2j+1 (rows 64-127)
    ktm = np.zeros((128, 128 * max(max_nb, 1)), np.float16)
    kT = key[b].T.astype(np.float16)  # [64, S]
    for j in range(max_nb):
        ktm[0:64, 128 * j : 128 * (j + 1)] = kT[:, 128 * (2 * j) : 128 * (2 * j + 1)]
        ktm[64:128, 128 * j : 128 * (j + 1)] = kT[
            :, 128 * (2 * j + 1) : 128 * (2 * j + 2)
        ]
    vaug = np.ones((S, 65), np.float16)
    vaug[:, :64] = value[b]
    # vm8: V pairs (all tiles), fp8e4m3, padded to stride 80
    vm8 = np.zeros((128, 2, W8), E4NP)
    v8 = vaug.astype(E4NP)  # RNE quantization
    for p in range(max_nb8):
        for j in range(2):
            t = 2 * p + j
            vm8[:, j, 80 * p : 80 * p + 65] = v8[128 * t : 128 * (t + 1), :]
    for m in range(4):
        c = _chunk_index(slot_c, m)
        n = slot_c[m]
        diag_first = m == 3 and n_shared[m] >= 4
        qchunk = query[b, 512 * c : 512 * (c + 1), :].T.astype(np.float16)
        qt[0:64, 512 * m : 512 * (m + 1)] = qchunk
        qt[64:128, 512 * m : 512 * (m + 1)] = qchunk
        for p in range(n_slab[m]):
            if diag_first:
                t = (n - 4 + p) if p < 4 else (n - 8 + (p - 4))
            else:
                t = n_shared[m] + p
            row = slice(0, 64) if p % 2 == 0 else slice(64, 128)
            col = slice(512 * m + 128 * (p // 2), 512 * m + 128 * (p // 2 + 1))
            kts[row, col] = key[b, 128 * t : 128 * (t + 1), :].T
        if n == 4:  # short-row chunk: fp16 V for tiles 0..3
            for g in range(4):
                vs[:, 65 * g : 65 * (g + 1)] = vaug[128 * g : 128 * (g + 1), :]
    mask = np.triu(np.ones((128, 512), dtype=np.float16))
    # mb: fused Schraudolph-bias masks for diag gangs, [128, 2, 1024]
    # variant v=0 (tiles g=0,1), v=1 (tiles g=2,3); scores sit at their true
    # q-columns, so cols < off are stale -> MASKB, the [off, off+128) block
    # is the causal triangle, and cols >= off+128 are fully allowed
    mb = np.full((128, 2, 1024), MASKB, np.float16)
    tri128 = np.triu(np.ones((128, 128), bool))
    for v in range(2):
        for j in range(2):
            off = 128 * (2 * v + j)
            blk = np.full((128, 512), MASKB, np.float16)
            blk[:, off : off + 128] = np.where(
                tri128, np.float16(SCH_B8), np.float16(MASKB)
            )
            blk[:, off + 128 :] = np.float16(SCH_B8)
            mb[:, j, 512 * v : 512 * (v + 1)] = blk
    return {
        "qt": qt, "ktm": ktm, "kts": kts, "vm8": vm8, "vs": vs,
        "mask": mask, "mb": mb,
    }


def _make_runner(nc, devices):
    """Vendored multi-core run_bass_via_pjrt with an explicit device set,
    split into an async dispatch and a blocking unpack."""
    from jax.sharding import Mesh, PartitionSpec

    bass2jax.install_neuronx_cc_hook()
    n = len(devices)
    partition_name = nc.partition_id_tensor.name if nc.partition_id_tensor else None
    in_names, out_names, out_avals, zero_outs = [], [], [], []
    for alloc in nc.m.functions[0].allocations:
        if not isinstance(alloc, mybir.MemoryLocationSet):
            continue
        name = alloc.memorylocations[0].name
        if alloc.kind == "ExternalInput":
            if name != partition_name:
                in_names.append(name)
        elif alloc.kind == "ExternalOutput":
            out_names.append(name)
            shape = tuple(alloc.tensor_shape)
            dtype = mybir.dt.np(alloc.dtype)
            out_avals.append(jax.core.ShapedArray(shape, dtype))
            zero_outs.append(np.zeros(shape, dtype))
    n_params = len(in_names)
    all_in = list(in_names) + list(out_names)
    if partition_name is not None:
        all_in.append(partition_name)
    all_in = tuple(all_in)
    donate = tuple(range(n_params, n_params + len(out_names)))

    def _body(*args):
        operands = list(args)
        if partition_name is not None:
            operands.append(bass2jax.partition_id_tensor())
        outs = bass2jax._bass_exec_p.bind(
            *operands,
            out_avals=tuple(out_avals),
            in_names=all_in,
            out_names=tuple(out_names),
            lowering_input_output_aliases=(),
            sim_require_finite=True,
            sim_require_nnan=True,
            nc=nc,
        )
        return tuple(outs)

    mesh = Mesh(np.asarray(devices), ("core",))
    in_specs = (PartitionSpec("core"),) * (n_params + len(out_names))
    out_specs = (PartitionSpec("core"),) * len(out_names)
    sharded = jax.jit(
        jax.shard_map(
            _body, mesh=mesh, in_specs=in_specs, out_specs=out_specs, check_vma=False
        ),
        donate_argnums=donate,
        keep_unused=True,
    )

    def dispatch(in_maps):
        concat_in = [
            np.concatenate([np.asarray(in_maps[c][nm]) for c in range(n)], axis=0)
            for nm in in_names
        ]
        concat_zeros = [
            np.zeros((n * z.shape[0], *z.shape[1:]), z.dtype) for z in zero_outs
        ]
        return sharded(*concat_in, *concat_zeros)

    def unpack(out_arrs):
        return [
            {
                nm: np.asarray(out_arrs[i]).reshape(n, *out_avals[i].shape)[c]
                for i, nm in enumerate(out_names)
            }
            for c in range(n)
        ]

    return dispatch, unpack


def _get_engine():
    if "engine" not in _cache:
        devs = jax.devices()
        ncA = _build_program(SLOT_A, 10, act_seed=0.0)
        ncB = _build_program(SLOT_B, 10, act_seed=0.0)
        dispA, unpackA = _make_runner(ncA, devs[0:4])
        dispB, unpackB = _make_runner(ncB, devs[4:8])
        _cache["engine"] = (dispA, unpackA, dispB, unpackB)
        _cache["ncs"] = (ncA, ncB)
    return _cache["engine"]


def run(query, key, value):
    dispA, unpackA, dispB, unpackB = _get_engine()
    mapsA = [_prep_core_inputs(SLOT_A, b, query, key, value) for b in range(4)]
    mapsB = [_prep_core_inputs(SLOT_B, b, query, key, value) for b in range(4)]
    outA = dispA(mapsA)
    outB = dispB(mapsB)
    resA = unpackA(outA)
    resB = unpackB(outB)

    out = np.zeros((B, S, D), np.float32)
    for b in range(4):
        for slot_c, res in ((SLOT_A, resA[b]), (SLOT_B, resB[b])):
            o = res["o"]  # [65, 2048]: chunk m at cols [512m, 512m+512)
            for m in range(4):
                c = _chunk_index(slot_c, m)
                blk = o[:, 512 * m : 512 * (m + 1)]
                out[b, 512 * c : 512 * (c + 1), :] = (blk[:64] / blk[64]).T
    return out


def kernel(query, key, value):
    query = np.ascontiguousarray(np.asarray(query, dtype=np.float32))
    key = np.ascontiguousarray(np.asarray(key, dtype=np.float32))
    value = np.ascontiguousarray(np.asarray(value, dtype=np.float32))
    return run(query, key, value)


# revision 53
# speedup vs baseline: 1.0788x; 1.0425x over previous
"""Causal attention (B=4, S=4096, D=64, fp32) on 8 Trainium2 NeuronCores.

v3c: all gangs except the short-row chunk run fp8e4m3 DoubleRow PV —
one matmul per 2-tile gang contracting 256 keys at 2 MACs/cell/cycle.
Logits are shifted by C=2 before exp (softmax-invariant) so
unnormalized probs fit e4m3's +-240 range. The exp fans out over THREE
engines: ACT (exact exp -> fp8, bit-exact RNE), DVE and GPSIMD
(Schraudolph affine -> saturating uint8 viewed as fp8e4m3; negatives
clamp to +0). Diagonal gangs fuse the causal mask into the Schraudolph
bias tensor (scalar_tensor_tensor with -1000 on masked lanes -> byte
saturates to +0), eliminating the separate mask multiply. The
short-row chunk (C=4) keeps the exact fp16 path.

Layout as v2: scores transposed S^T[k,q] (d on partitions), QK packed
2-up on the PE via row-group tiling, normalization deferred via a
ones-column in V accumulating row sums.
"""

import numpy as np
import ml_dtypes

import jax
import concourse.bass as bass  # noqa: F401
import concourse.mybir as mybir
from concourse import bacc
from concourse import bass2jax
from concourse.tile import TileContext

B, S, D = 4, 4096, 64
NCORES = 8
SLOT_A = (12, 4, 24, 32)  # program A: chunks {2,0,5,7} of a batch (72 tiles)
SLOT_B = (8, 16, 20, 28)  # program B: chunks {1,3,4,6} (72 tiles)
F32 = mybir.dt.float32
F16 = mybir.dt.float16
F8 = mybir.dt.float8e4
U8 = mybir.dt.uint8
I16 = mybir.dt.int16
E4NP = ml_dtypes.float8_e4m3

LOG2E = 1.4426950408889634
CSH = 2.0  # logit shift: p = exp(score/8 - CSH), softmax-invariant
# fp16 Schraudolph (int16 bit trick), shift folded into the bias
SCH_A = 0.125 * 1024 * LOG2E
SCH_B = (15.0 - 0.0435) * 1024.0 - CSH * 1024.0 * LOG2E
# fp8 Schraudolph (saturating uint8 viewed as e4m3), shift in bias
SCH_A8 = 0.125 * 8.0 * LOG2E
SCH_B8 = 56.0 - CSH * 8.0 * LOG2E - 0.40
MASKB = -1000.0

_cache = {}


def _chunk_index(slot_c, m):
    return slot_c[m] // 4 - 1


def _build_program(slot_c, warmup_n, act_seed=900.0, split_qt=False):
    n_shared = [max(c - 8, 0) for c in slot_c]
    n_slab = [min(c, 8) for c in slot_c]
    nb_shared = [ns // 2 for ns in n_shared]
    n_pr = [c // 2 for c in slot_c]  # V pairs per chunk (all tiles)
    max_nb = max(nb_shared)
    max_nb8 = max(n_pr)
    W8 = 80 * max_nb8

    nc = bacc.Bacc("TRN2", target_bir_lowering=False, debug=False)
    qt_d = nc.declare_dram_parameter("qt", [128, 2048], F16, isOutput=False)
    ktm_d = nc.declare_dram_parameter(
        "ktm", [128, 128 * max(max_nb, 1)], F16, isOutput=False
    )
    kts_d = nc.declare_dram_parameter("kts", [128, 2048], F16, isOutput=False)
    vm8_d = nc.declare_dram_parameter("vm8", [128, 2, W8], F8, isOutput=False)
    vs_d = nc.declare_dram_parameter("vs", [128, 260], F16, isOutput=False)
    mk_d = nc.declare_dram_parameter("mask", [128, 512], F16, isOutput=False)
    mb_d = nc.declare_dram_parameter("mb", [128, 2, 1024], F16, isOutput=False)
    o_d = nc.declare_dram_parameter("o", [65, 2048], F32, isOutput=True)
    EXP = mybir.ActivationFunctionType.Exp
    MUL = mybir.AluOpType.mult
    ADD = mybir.AluOpType.add

    # ---- exp engine plan (greedy balance, built at trace time) ----
    act_t, dve_t, gp_t = [act_seed], [0.0], [0.0]

    def plan_exp(cols, allow=("act", "dve")):
        costs = {
            "act": 0.87 * cols + 90.0,
            "dve": 1.0417 * cols + 70.0,
            "gp": 1.39 * cols + 265.0,
        }
        tot = {"act": act_t[0], "dve": dve_t[0], "gp": gp_t[0]}
        eng = min(allow, key=lambda e: tot[e] + costs[e])
        if eng == "act":
            act_t[0] += costs["act"]
        elif eng == "dve":
            dve_t[0] += costs["dve"]
        else:
            gp_t[0] += costs["gp"]
        return eng

    with TileContext(nc) as tc:
        with (
            tc.tile_pool(name="cons", bufs=1) as cons,
            tc.tile_pool(name="data", bufs=1) as data,
            tc.tile_pool(name="pp", bufs=5) as pp,
            tc.tile_pool(name="ep", bufs=2) as ep,
            tc.tile_pool(name="ps_sc", bufs=3, space="PSUM") as ps_sc,
            tc.tile_pool(name="ps_acc", bufs=2, space="PSUM") as ps_acc,
        ):
            warm = cons.tile([128, 512], F16)
            biasC = cons.tile([128, 1], F32)
            dummy = cons.tile([128, 1], F32)
            nc.vector.memset(warm[:, 0:256], 0.0)
            nc.gpsimd.memset(warm[:, 256:512], 0.0)
            nc.vector.memset(biasC[:], -CSH)
            # pull ACT_TABLE_LOAD to the front of the Scalar queue so the
            # first real exp isn't serialized behind a 1.3us table load
            nc.scalar.activation(
                dummy[:], biasC[:], EXP, scale=1.0, bias=biasC[:]
            )
            for w in range(warmup_n):
                # alternate subtiles so every sc byte is finite before any
                # full-width diag read (uninit PSUM can be NaN)
                wp = ps_sc.tile([128, 2, 512], F32, tag="sc")
                nc.tensor.matmul(
                    wp[:, w % 2, :], warm[:, 0:128], warm[:], start=True, stop=True
                )

            qt = data.tile([128, 2048], F16)
            kts = data.tile([128, 2048], F16)
            vs = data.tile([128, 260], F16)
            ktm = data.tile([128, 128 * max(max_nb, 1)], F16)
            vm8 = data.tile([128, 2, W8], F8)
            mb = cons.tile([128, 2, 1024], F16)  # [:, :, 0:512]=mb01, 512:1024=mb23

            def dma_slot(m):
                nsb = n_slab[m] // 2
                nc.sync.dma_start(
                    out=kts[:, 512 * m : 512 * m + 128 * nsb],
                    in_=kts_d[:, 512 * m : 512 * m + 128 * nsb],
                )
                if slot_c[m] == 4:
                    nc.sync.dma_start(out=vs[:], in_=vs_d[:])

            def dma_k(lo, hi):  # shared k blocks (pairs) [lo, hi)
                if hi <= lo:
                    return
                nc.sync.dma_start(
                    out=ktm[:, 128 * lo : 128 * hi], in_=ktm_d[:, 128 * lo : 128 * hi]
                )

            def dma_v8(lo, hi):  # V pairs [lo, hi)
                if hi <= lo:
                    return
                nc.sync.dma_start(
                    out=vm8[:, :, 80 * lo : 80 * hi], in_=vm8_d[:, :, 80 * lo : 80 * hi]
                )

            if not split_qt:
                nc.sync.dma_start(out=qt[:], in_=qt_d[:])
            mask = cons.tile([128, 512], F16)
            done_k = 0
            done_v = 0
            for m in range(4):
                if split_qt:
                    # per-chunk qt piece, then K data, then V data
                    nc.sync.dma_start(
                        out=qt[:, 512 * m : 512 * (m + 1)],
                        in_=qt_d[:, 512 * m : 512 * (m + 1)],
                    )
                    while done_k < nb_shared[m]:
                        step = min(3, nb_shared[m] - done_k)
                        dma_k(done_k, done_k + step)
                        done_k += step
                    dma_slot(m)
                    while done_v < n_pr[m]:
                        step = min(4, n_pr[m] - done_v)
                        dma_v8(done_v, done_v + step)
                        done_v += step
                else:
                    while done_k < nb_shared[m]:
                        step = min(3, nb_shared[m] - done_k)
                        dma_k(done_k, done_k + step)
                        done_k += step
                    while done_v < n_pr[m]:
                        step = min(4, n_pr[m] - done_v)
                        dma_v8(done_v, done_v + step)
                        done_v += step
                    dma_slot(m)
                if m == 0:
                    nc.sync.dma_start(out=mask[:], in_=mk_d[:])
                    nc.sync.dma_start(out=mb[:], in_=mb_d[:])
                fut_k = max(nb_shared[m:])
                if done_k < fut_k:
                    step = min(3, fut_k - done_k)
                    dma_k(done_k, done_k + step)
                    done_k += step
                fut_v = max(n_pr[m:])
                if done_v < fut_v:
                    step = min(4, fut_v - done_v)
                    dma_v8(done_v, done_v + step)
                    done_v += step

            pending = []  # (emit_fn, pt, gang, after_fn) across chunks
            gang_no = [0]  # global gang counter (plug placement)
            dr_no = [0]  # global DR-gang counter (fill split)

            def pump(limit):
                while len(pending) > limit:
                    fn, pt_, gang_, after = pending.pop(0)
                    fn(pt_, gang_)
                    if after is not None:
                        after()

            for m in range(4):
                C = slot_c[m]
                ns = n_shared[m]
                diag_first = m == 3 and ns >= 4
                short_rows = C == 4  # chunk 0: exact fp16 path
                q_sl = slice(512 * m, 512 * (m + 1))
                acc = ps_acc.tile([65, 512], F32, tag="acc")

                def tile_geom(t, C=C, ns=ns, diag_first=diag_first):
                    g = (t - ns) if diag_first else (t - (C - 4))
                    if 0 <= g <= 3:
                        off = 128 * g
                    else:
                        off = 0
                    return (g if 0 <= g <= 3 else -1), off, 512 - off

                def dr_pair(t0, C=C, ns=ns, diag_first=diag_first):
                    # vm8 pair index for the gang starting at position t0
                    if diag_first:
                        if ns <= t0 < ns + 4:
                            return (C - 4 + (t0 - ns)) // 2
                        if t0 >= ns + 4:
                            return (t0 - 4) // 2
                    return t0 // 2

                def emit_pv(
                    pt, gang, C=C, m=m, acc=acc,
                    tile_geom=tile_geom, dr_pair=dr_pair, short_rows=short_rows,
                ):
                    t0 = gang[0]
                    if not short_rows:
                        # fp8 DoubleRow matmul for both tiles (diag incl.)
                        pr = dr_pair(t0)
                        pt8 = pt.bitcast(F8)
                        nc.tensor.matmul(
                            acc[:],
                            vm8[:, :, 80 * pr : 80 * pr + 65],
                            pt8[:, :, 0:512],
                            start=(t0 == 0),
                            stop=(gang[1] == C - 1),
                            perf_mode=mybir.MatmulPerfMode.DoubleRow,
                        )
                        return
                    for j, t in enumerate(gang):
                        g, off, w = tile_geom(t)
                        ptile = pt[:, j, 0:w]
                        nc.vector.tensor_mul(ptile, ptile, mask[:, :w])
                        vt = vs[:, 65 * g : 65 * (g + 1)]
                        nc.tensor.matmul(
                            acc[:, off:512],
                            vt,
                            ptile,
                            start=(t == 0),
                            stop=(t == C - 1),
                        )

                def make_epilogue(m=m, acc=acc, last=(m == 3)):
                    def epilogue():
                        osb = ep.tile([65, 512], F32, tag="osb")
                        if last:
                            # final chunk: copy+DMA in two halves on DVE so
                            # the first half's DMA overlaps the second copy
                            nc.vector.tensor_copy(osb[:, 0:256], acc[:, 0:256])
                            nc.sync.dma_start(
                                out=o_d[:, 512 * m : 512 * m + 256],
                                in_=osb[:, 0:256],
                            )
                            nc.vector.tensor_copy(
                                osb[:, 256:512], acc[:, 256:512]
                            )
                            dve_t[0] += 800.0
                        else:
                            # split the PSUM->SBUF copy across DVE and ACT
                            nc.vector.tensor_copy(osb[:, 0:256], acc[:, 0:256])
                            nc.scalar.activation(
                                osb[:, 256:512],
                                acc[:, 256:512],
                                mybir.ActivationFunctionType.Copy,
                            )
                            act_t[0] += 360.0
                            dve_t[0] += 360.0
                        if last:
                            nc.sync.dma_start(
                                out=o_d[:, 512 * m + 256 : 512 * (m + 1)],
                                in_=osb[:, 256:512],
                            )
                        else:
                            nc.sync.dma_start(
                                out=o_d[:, 512 * m : 512 * (m + 1)], in_=osb[:]
                            )

                    return epilogue

                n_gangs = (C + 1) // 2
                positions = list(range(0, C, 2))
                if not short_rows and C >= 12:
                    # spread the two diag gangs (forced-DVE stt) between DR
                    # gangs so ACT isn't starved at chunk tails
                    if diag_first:
                        i = positions.index(ns)
                        positions[i + 1], positions[i + 2] = (
                            positions[i + 2], positions[i + 1],
                        )
                    else:
                        positions[-3], positions[-2] = (
                            positions[-2], positions[-3],
                        )
                for gi, t0 in enumerate(positions):
                    gang = (t0, t0 + 1)
                    sc = ps_sc.tile([128, 2, 512], F32, tag="sc")
                    geoms = []
                    for j, t in enumerate(gang):
                        g, off, w = tile_geom(t)
                        geoms.append((g, off, w))
                        if t < ns:
                            blk = t // 2
                            lhsT = ktm[
                                64 * j : 64 * (j + 1), 128 * blk : 128 * (blk + 1)
                            ]
                        else:
                            p = t - ns
                            blk = p // 2
                            lhsT = kts[
                                64 * j : 64 * (j + 1),
                                512 * m + 128 * blk : 512 * m + 128 * (blk + 1),
                            ]
                        rhs = qt[64 * j : 64 * (j + 1), q_sl]
                        if off:
                            rhs = rhs[:, off:512]
                        # diag scores land at their true q-columns so the
                        # full-width DoubleRow PV accumulates them unshifted;
                        # the short-row fp16 path keeps the 0-based window
                        dst = (
                            sc[:, j, 0:w] if short_rows else sc[:, j, off:512]
                        )
                        nc.tensor.matmul(
                            dst,
                            lhsT,
                            rhs,
                            start=True,
                            stop=True,
                        )
                    pt = pp.tile([128, 2, 512], F16, tag="pt")
                    is_diag = any(g >= 0 for g, _, _ in geoms)
                    if short_rows:
                        # fp16 path, mask applied via tensor_mul later.
                        # Even tiles exact on ACT (covers the shortest rows);
                        # odd tiles Schraudolph-fp16 on DVE (rows there have
                        # >=129 keys, enough wash-out).
                        pti = pt.bitcast(I16)
                        for j, (g, off, w) in enumerate(geoms):
                            if gang[j] % 2 == 0:
                                nc.scalar.activation(
                                    pt[:, j, 0:w], sc[:, j, 0:w], EXP,
                                    scale=0.125, bias=biasC[:],
                                )
                                act_t[0] += 0.87 * w + 90.0
                            else:
                                nc.vector.tensor_scalar(
                                    pti[:, j, 0:w], sc[:, j, 0:w],
                                    SCH_A, SCH_B, MUL, ADD,
                                )
                                dve_t[0] += 1.0417 * w + 70.0
                        dve_t[0] += 0.52 * sum(w for _, _, w in geoms) + 300
                    elif is_diag:
                        # fused exp+mask Schraudolph -> fp8 on DVE, narrowed
                        # to the valid window; GPSIMD zeroes the stale cols
                        ptu8 = pt.bitcast(U8)
                        v = 0 if geoms[0][0] == 0 else 1  # mb01 or mb23
                        for j, (g, off, w) in enumerate(geoms):
                            if off:
                                nc.gpsimd.memset(ptu8[:, j, 0:off], 0)
                            mbs = mb[:, j, 512 * v + off : 512 * (v + 1)]
                            plan_exp(w, allow=("dve",))
                            nc.vector.scalar_tensor_tensor(
                                ptu8[:, j, off:512], sc[:, j, off:512],
                                SCH_A8, mbs, MUL, ADD,
                            )
                    else:
                        pt8 = pt.bitcast(F8)
                        ptu8 = pt.bitcast(U8)
                        if dr_no[0] < 3:
                            # pipeline fill: split first gangs across engines
                            nc.scalar.activation(
                                pt8[:, 0, 0:512], sc[:, 0, :], EXP,
                                scale=0.125, bias=biasC[:],
                            )
                            nc.vector.tensor_scalar(
                                ptu8[:, 1, 0:512], sc[:, 1, :],
                                SCH_A8, SCH_B8, MUL, ADD,
                            )
                            act_t[0] += 0.87 * 512 + 90.0
                            dve_t[0] += 1.0417 * 512 + 70.0
                        else:
                            # tail: keep DVE free for the final copy chain
                            force = (
                                ("act",)
                                if (m == 3 and gi >= n_gangs - 2)
                                else ("act", "dve")
                            )
                            eng = plan_exp(1024, allow=force)
                            if eng == "act":
                                nc.scalar.activation(
                                    pt8[:, :, 0:512], sc[:], EXP,
                                    scale=0.125, bias=biasC[:],
                                )
                            else:
                                nc.vector.tensor_scalar(
                                    ptu8[:, :, 0:512], sc[:],
                                    SCH_A8, SCH_B8, MUL, ADD,
                                )
                    after = make_epilogue() if gi == n_gangs - 1 else None
                    pending.append((emit_pv, pt, gang, after))
                    if not is_diag and not short_rows:
                        dr_no[0] += 1
                    gang_no[0] += 1
                    # plugs must stop before the first start=False PV pop
                    # (they'd clobber live accumulation in the acc bufs)
                    if gang_no[0] <= 3:
                        # fill plugs: dependency-free PE work in the acc bank
                        # keeps the PE dense (HAM-warm) while the first
                        # gangs' exp results are still in flight (the next
                        # real PV's start=True clears the plug garbage)
                        for _ in range(2 + (gang_no[0] <= 2)):
                            wpa = ps_acc.tile([65, 512], F32, tag="acc")
                            nc.tensor.matmul(
                                wpa[:], warm[:, 0:65], warm[:],
                                start=True, stop=True,
                            )
                    pump(2 if m == 0 else 3)
            pump(0)

    nc.compile()
    return nc


def _prep_core_inputs(slot_c, b, query, key, value):
    n_shared = [max(c - 8, 0) for c in slot_c]
    n_slab = [min(c, 8) for c in slot_c]
    nb_shared = [ns // 2 for ns in n_shared]
    n_pr = [c // 2 for c in slot_c]
    max_nb = max(nb_shared)
    max_nb8 = max(n_pr)
    W8 = 80 * max_nb8

    qt = np.zeros((128, 2048), np.float16)
    kts = np.zeros((128, 2048), np.float16)
    vs = np.zeros((128, 260), np.float16)
    # ktm: block j holds tiles 2j (rows 0-63) and 2j+1 (rows 64-127)
    ktm = np.zeros((128, 128 * max(max_nb, 1)), np.float16)
    kT = key[b].T.astype(np.float16)  # [64, S]
    for j in range(max_nb):
        ktm[0:64, 128 * j : 128 * (j + 1)] = kT[:, 128 * (2 * j) : 128 * (2 * j + 1)]
        ktm[64:128, 128 * j : 128 * (j + 1)] = kT[
            :, 128 * (2 * j + 1) : 128 * (2 * j + 2)
        ]
    vaug = np.ones((S, 65), np.float16)
    vaug[:, :64] = value[b]
    # vm8: V pairs (all tiles), fp8e4m3, padded to stride 80
    vm8 = np.zeros((128, 2, W8), E4NP)
    v8 = vaug.astype(E4NP)  # RNE quantization
    for p in range(max_nb8):
        for j in range(2):
            t = 2 * p + j
            vm8[:, j, 80 * p : 80 * p + 65] = v8[128 * t : 128 * (t + 1), :]
    for m in range(4):
        c = _chunk_index(slot_c, m)
        n = slot_c[m]
        diag_first = m == 3 and n_shared[m] >= 4
        qchunk = query[b, 512 * c : 512 * (c + 1), :].T.astype(np.float16)
        qt[0:64, 512 * m : 512 * (m + 1)] = qchunk
        qt[64:128, 512 * m : 512 * (m + 1)] = qchunk
        for p in range(n_slab[m]):
            if diag_first:
                t = (n - 4 + p) if p < 4 else (n - 8 + (p - 4))
            else:
                t = n_shared[m] + p
            row = slice(0, 64) if p % 2 == 0 else slice(64, 128)
            col = slice(512 * m + 128 * (p // 2), 512 * m + 128 * (p // 2 + 1))
            kts[row, col] = key[b, 128 * t : 128 * (t + 1), :].T
        if n == 4:  # short-row chunk: fp16 V for tiles 0..3
            for g in range(4):
                vs[:, 65 * g : 65 * (g + 1)] = vaug[128 * g : 128 * (g + 1), :]
    mask = np.triu(np.ones((128, 512), dtype=np.float16))
    # mb: fused Schraudolph-bias masks for diag gangs, [128, 2, 1024]
    # variant v=0 (tiles g=0,1), v=1 (tiles g=2,3); scores sit at their true
    # q-columns, so cols < off are stale -> MASKB, the [off, off+128) block
    # is the causal triangle, and cols >= off+128 are fully allowed
    mb = np.full((128, 2, 1024), MASKB, np.float16)
    tri128 = np.triu(np.ones((128, 128), bool))
    for v in range(2):
        for j in range(2):
            off = 128 * (2 * v + j)
            blk = np.full((128, 512), MASKB, np.float16)
            blk[:, off : off + 128] = np.where(
                tri128, np.float16(SCH_B8), np.float16(MASKB)
            )
            blk[:, off + 128 :] = np.float16(SCH_B8)
            mb[:, j, 512 * v : 512 * (v + 1)] = blk
    return {
        "qt": qt, "ktm": ktm, "kts": kts, "vm8": vm8, "vs": vs,
        "mask": mask, "mb": mb,
    }


def _make_runner(nc, devices):
    """Vendored multi-core run_bass_via_pjrt with an explicit device set,
    split into an async dispatch and a blocking unpack."""
    from jax.sharding import Mesh, PartitionSpec

    bass2jax.install_neuronx_cc_hook()
    n = len(devices)
    partition_name = nc.partition_id_tensor.name if nc.partition_id_tensor else None
    in_names, out_names, out_avals, zero_outs = [], [], [], []
    for alloc in nc.m.functions[0].allocations:
        if not isinstance(alloc, mybir.MemoryLocationSet):
            continue
        name = alloc.memorylocations[0].name
        if alloc.kind == "ExternalInput":
            if name != partition_name:
                in_names.append(name)
        elif alloc.kind == "ExternalOutput":
            out_names.append(name)
            shape = tuple(alloc.tensor_shape)
            dtype = mybir.dt.np(alloc.dtype)
            out_avals.append(jax.core.ShapedArray(shape, dtype))
            zero_outs.append(np.zeros(shape, dtype))
    n_params = len(in_names)
    all_in = list(in_names) + list(out_names)
    if partition_name is not None:
        all_in.append(partition_name)
    all_in = tuple(all_in)
    donate = tuple(range(n_params, n_params + len(out_names)))

    def _body(*args):
        operands = list(args)
        if partition_name is not None:
            operands.append(bass2jax.partition_id_tensor())
        outs = bass2jax._bass_exec_p.bind(
            *operands,
            out_avals=tuple(out_avals),
            in_names=all_in,
            out_names=tuple(out_names),
            lowering_input_output_aliases=(),
            sim_require_finite=True,
            sim_require_nnan=True,
            nc=nc,
        )
        return tuple(outs)

    mesh = Mesh(np.asarray(devices), ("core",))
    in_specs = (PartitionSpec("core"),) * (n_params + len(out_names))
    out_specs = (PartitionSpec("core"),) * len(out_names)
    sharded = jax.jit(
        jax.shard_map(
            _body, mesh=mesh, in_specs=in_specs, out_specs=out_specs, check_vma=False
        ),
        donate_argnums=donate,
        keep_unused=True,
    )

    def dispatch(in_maps):
        concat_in = [
            np.concatenate([np.asarray(in_maps[c][nm]) for c in range(n)], axis=0)
            for nm in in_names
        ]
        concat_zeros = [
            np.zeros((n * z.shape[0], *z.shape[1:]), z.dtype) for z in zero_outs
        ]
        return sharded(*concat_in, *concat_zeros)

    def unpack(out_arrs):
        return [
            {
                nm: np.asarray(out_arrs[i]).reshape(n, *out_avals[i].shape)[c]
                for i, nm in enumerate(out_names)
            }
            for c in range(n)
        ]

    return dispatch, unpack


def _get_engine():
    if "engine" not in _cache:
        devs = jax.devices()
        ncA = _build_program(SLOT_A, 10, act_seed=0.0)
        ncB = _build_program(SLOT_B, 7, act_seed=0.0, split_qt=True)
        dispA, unpackA = _make_runner(ncA, devs[0:4])
        dispB, unpackB = _make_runner(ncB, devs[4:8])
        _cache["engine"] = (dispA, unpackA, dispB, unpackB)
        _cache["ncs"] = (ncA, ncB)
    return _cache["engine"]


def run(query, key, value):
    dispA, unpackA, dispB, unpackB = _get_engine()
    mapsA = [_prep_core_inputs(SLOT_A, b, query, key, value) for b in range(4)]
    mapsB = [_prep_core_inputs(SLOT_B, b, query, key, value) for b in range(4)]
    outA = dispA(mapsA)
    outB = dispB(mapsB)
    resA = unpackA(outA)
    resB = unpackB(outB)

    out = np.zeros((B, S, D), np.float32)
    for b in range(4):
        for slot_c, res in ((SLOT_A, resA[b]), (SLOT_B, resB[b])):
            o = res["o"]  # [65, 2048]: chunk m at cols [512m, 512m+512)
            for m in range(4):
                c = _chunk_index(slot_c, m)
                blk = o[:, 512 * m : 512 * (m + 1)]
                out[b, 512 * c : 512 * (c + 1), :] = (blk[:64] / blk[64]).T
    return out


def kernel(query, key, value):
    query = np.ascontiguousarray(np.asarray(query, dtype=np.float32))
    key = np.ascontiguousarray(np.asarray(key, dtype=np.float32))
    value = np.ascontiguousarray(np.asarray(value, dtype=np.float32))
    return run(query, key, value)


# revision 55
# speedup vs baseline: 1.0892x; 1.0096x over previous
"""Causal attention (B=4, S=4096, D=64, fp32) on 8 Trainium2 NeuronCores.

v3c: all gangs except the short-row chunk run fp8e4m3 DoubleRow PV —
one matmul per 2-tile gang contracting 256 keys at 2 MACs/cell/cycle.
Logits are shifted by C=2 before exp (softmax-invariant) so
unnormalized probs fit e4m3's +-240 range. The exp fans out over THREE
engines: ACT (exact exp -> fp8, bit-exact RNE), DVE and GPSIMD
(Schraudolph affine -> saturating uint8 viewed as fp8e4m3; negatives
clamp to +0). Diagonal gangs fuse the causal mask into the Schraudolph
bias tensor (scalar_tensor_tensor with -1000 on masked lanes -> byte
saturates to +0), eliminating the separate mask multiply. The
short-row chunk (C=4) keeps the exact fp16 path.

Layout as v2: scores transposed S^T[k,q] (d on partitions), QK packed
2-up on the PE via row-group tiling, normalization deferred via a
ones-column in V accumulating row sums.
"""

import numpy as np
import ml_dtypes

import jax
import concourse.bass as bass  # noqa: F401
import concourse.mybir as mybir
from concourse import bacc
from concourse import bass2jax
from concourse.tile import TileContext

B, S, D = 4, 4096, 64
NCORES = 8
SLOT_A = (12, 4, 24, 32)  # program A: chunks {2,0,5,7} of a batch (72 tiles)
SLOT_B = (8, 16, 20, 28)  # program B: chunks {1,3,4,6} (72 tiles)
F32 = mybir.dt.float32
F16 = mybir.dt.float16
F8 = mybir.dt.float8e4
U8 = mybir.dt.uint8
I16 = mybir.dt.int16
E4NP = ml_dtypes.float8_e4m3

LOG2E = 1.4426950408889634
CSH = 2.0  # logit shift: p = exp(score/8 - CSH), softmax-invariant
# fp16 Schraudolph (int16 bit trick), shift folded into the bias
SCH_A = 0.125 * 1024 * LOG2E
SCH_B = (15.0 - 0.0435) * 1024.0 - CSH * 1024.0 * LOG2E
# fp8 Schraudolph (saturating uint8 viewed as e4m3), shift in bias
SCH_A8 = 0.125 * 8.0 * LOG2E
SCH_B8 = 56.0 - CSH * 8.0 * LOG2E - 0.40
MASKB = -1000.0

_cache = {}


def _chunk_index(slot_c, m):
    return slot_c[m] // 4 - 1


def _build_program(slot_c, warmup_n, act_seed=900.0, split_qt=False):
    n_shared = [max(c - 8, 0) for c in slot_c]
    n_slab = [min(c, 8) for c in slot_c]
    nb_shared = [ns // 2 for ns in n_shared]
    n_pr = [c // 2 for c in slot_c]  # V pairs per chunk (all tiles)
    max_nb = max(nb_shared)
    max_nb8 = max(n_pr)
    W8 = 80 * max_nb8

    nc = bacc.Bacc("TRN2", target_bir_lowering=False, debug=False)
    qt_d = nc.declare_dram_parameter("qt", [128, 2048], F16, isOutput=False)
    ktm_d = nc.declare_dram_parameter(
        "ktm", [128, 128 * max(max_nb, 1)], F16, isOutput=False
    )
    kts_d = nc.declare_dram_parameter("kts", [128, 2048], F16, isOutput=False)
    vm8_d = nc.declare_dram_parameter("vm8", [128, 2, W8], F8, isOutput=False)
    vs_d = nc.declare_dram_parameter("vs", [128, 260], F16, isOutput=False)
    mk_d = nc.declare_dram_parameter("mask", [128, 512], F16, isOutput=False)
    mb_d = nc.declare_dram_parameter("mb", [128, 2, 1024], F16, isOutput=False)
    o_d = nc.declare_dram_parameter("o", [65, 2048], F32, isOutput=True)
    EXP = mybir.ActivationFunctionType.Exp
    MUL = mybir.AluOpType.mult
    ADD = mybir.AluOpType.add

    # ---- exp engine plan (greedy balance, built at trace time) ----
    act_t, dve_t, gp_t = [act_seed], [0.0], [0.0]

    def plan_exp(cols, allow=("act", "dve")):
        costs = {
            "act": 0.87 * cols + 90.0,
            "dve": 1.0417 * cols + 70.0,
            "gp": 1.39 * cols + 265.0,
        }
        tot = {"act": act_t[0], "dve": dve_t[0], "gp": gp_t[0]}
        eng = min(allow, key=lambda e: tot[e] + costs[e])
        if eng == "act":
            act_t[0] += costs["act"]
        elif eng == "dve":
            dve_t[0] += costs["dve"]
        else:
            gp_t[0] += costs["gp"]
        return eng

    with TileContext(nc) as tc:
        with (
            tc.tile_pool(name="cons", bufs=1) as cons,
            tc.tile_pool(name="data", bufs=1) as data,
            tc.tile_pool(name="pp", bufs=5) as pp,
            tc.tile_pool(name="ep", bufs=2) as ep,
            tc.tile_pool(name="ps_sc", bufs=3, space="PSUM") as ps_sc,
            tc.tile_pool(name="ps_acc", bufs=2, space="PSUM") as ps_acc,
        ):
            warm = cons.tile([128, 512], F16)
            biasC = cons.tile([128, 1], F32)
            dummy = cons.tile([128, 1], F32)
            nc.vector.memset(warm[:, 0:256], 0.0)
            nc.gpsimd.memset(warm[:, 256:512], 0.0)
            nc.vector.memset(biasC[:], -CSH)
            # pull ACT_TABLE_LOAD to the front of the Scalar queue so the
            # first real exp isn't serialized behind a 1.3us table load
            nc.scalar.activation(
                dummy[:], biasC[:], EXP, scale=1.0, bias=biasC[:]
            )
            for w in range(warmup_n):
                # alternate subtiles so every sc byte is finite before any
                # full-width diag read (uninit PSUM can be NaN)
                wp = ps_sc.tile([128, 2, 512], F32, tag="sc")
                nc.tensor.matmul(
                    wp[:, w % 2, :], warm[:, 0:128], warm[:], start=True, stop=True
                )

            qt = data.tile([128, 2048], F16)
            kts = data.tile([128, 2048], F16)
            vs = data.tile([128, 260], F16)
            ktm = data.tile([128, 128 * max(max_nb, 1)], F16)
            vm8 = data.tile([128, 2, W8], F8)
            mb = cons.tile([128, 2, 1024], F16)  # [:, :, 0:512]=mb01, 512:1024=mb23

            def dma_slot(m):
                nsb = n_slab[m] // 2
                nc.sync.dma_start(
                    out=kts[:, 512 * m : 512 * m + 128 * nsb],
                    in_=kts_d[:, 512 * m : 512 * m + 128 * nsb],
                )
                if slot_c[m] == 4:
                    nc.sync.dma_start(out=vs[:], in_=vs_d[:])

            def dma_k(lo, hi):  # shared k blocks (pairs) [lo, hi)
                if hi <= lo:
                    return
                nc.sync.dma_start(
                    out=ktm[:, 128 * lo : 128 * hi], in_=ktm_d[:, 128 * lo : 128 * hi]
                )

            def dma_v8(lo, hi):  # V pairs [lo, hi)
                if hi <= lo:
                    return
                nc.sync.dma_start(
                    out=vm8[:, :, 80 * lo : 80 * hi], in_=vm8_d[:, :, 80 * lo : 80 * hi]
                )

            if not split_qt:
                nc.sync.dma_start(out=qt[:], in_=qt_d[:])
            mask = cons.tile([128, 512], F16)
            done_k = 0
            done_v = 0
            for m in range(4):
                if split_qt:
                    # per-chunk qt piece, then K data, then V data
                    nc.sync.dma_start(
                        out=qt[:, 512 * m : 512 * (m + 1)],
                        in_=qt_d[:, 512 * m : 512 * (m + 1)],
                    )
                    while done_k < nb_shared[m]:
                        step = min(3, nb_shared[m] - done_k)
                        dma_k(done_k, done_k + step)
                        done_k += step
                    dma_slot(m)
                    while done_v < n_pr[m]:
                        step = min(4, n_pr[m] - done_v)
                        dma_v8(done_v, done_v + step)
                        done_v += step
                else:
                    while done_k < nb_shared[m]:
                        step = min(3, nb_shared[m] - done_k)
                        dma_k(done_k, done_k + step)
                        done_k += step
                    while done_v < n_pr[m]:
                        step = min(4, n_pr[m] - done_v)
                        dma_v8(done_v, done_v + step)
                        done_v += step
                    dma_slot(m)
                if m == 0:
                    nc.sync.dma_start(out=mask[:], in_=mk_d[:])
                    nc.sync.dma_start(out=mb[:], in_=mb_d[:])
                fut_k = max(nb_shared[m:])
                if done_k < fut_k:
                    step = min(3, fut_k - done_k)
                    dma_k(done_k, done_k + step)
                    done_k += step
                fut_v = max(n_pr[m:])
                if done_v < fut_v:
                    step = min(4, fut_v - done_v)
                    dma_v8(done_v, done_v + step)
                    done_v += step

            pending = []  # (emit_fn, pt, gang, after_fn) across chunks
            gang_no = [0]  # global gang counter (plug placement)
            dr_no = [0]  # global DR-gang counter (fill split)

            def pump(limit):
                while len(pending) > limit:
                    fn, pt_, gang_, after = pending.pop(0)
                    fn(pt_, gang_)
                    if after is not None:
                        after()

            for m in range(4):
                C = slot_c[m]
                ns = n_shared[m]
                diag_first = m == 3 and ns >= 4
                short_rows = C == 4  # chunk 0: exact fp16 path
                q_sl = slice(512 * m, 512 * (m + 1))
                acc = ps_acc.tile([65, 512], F32, tag="acc")

                def tile_geom(t, C=C, ns=ns, diag_first=diag_first):
                    g = (t - ns) if diag_first else (t - (C - 4))
                    if 0 <= g <= 3:
                        off = 128 * g
                    else:
                        off = 0
                    return (g if 0 <= g <= 3 else -1), off, 512 - off

                def dr_pair(t0, C=C, ns=ns, diag_first=diag_first):
                    # vm8 pair index for the gang starting at position t0
                    if diag_first:
                        if ns <= t0 < ns + 4:
                            return (C - 4 + (t0 - ns)) // 2
                        if t0 >= ns + 4:
                            return (t0 - 4) // 2
                    return t0 // 2

                def emit_pv(
                    pt, gang, C=C, m=m, acc=acc,
                    tile_geom=tile_geom, dr_pair=dr_pair, short_rows=short_rows,
                ):
                    t0 = gang[0]
                    if not short_rows:
                        # fp8 DoubleRow matmul for both tiles (diag incl.)
                        pr = dr_pair(t0)
                        pt8 = pt.bitcast(F8)
                        nc.tensor.matmul(
                            acc[:],
                            vm8[:, :, 80 * pr : 80 * pr + 65],
                            pt8[:, :, 0:512],
                            start=(t0 == 0),
                            stop=(gang[1] == C - 1),
                            perf_mode=mybir.MatmulPerfMode.DoubleRow,
                        )
                        return
                    for j, t in enumerate(gang):
                        g, off, w = tile_geom(t)
                        ptile = pt[:, j, 0:w]
                        nc.vector.tensor_mul(ptile, ptile, mask[:, :w])
                        vt = vs[:, 65 * g : 65 * (g + 1)]
                        nc.tensor.matmul(
                            acc[:, off:512],
                            vt,
                            ptile,
                            start=(t == 0),
                            stop=(t == C - 1),
                        )

                def make_epilogue(m=m, acc=acc, last=(m == 3)):
                    def epilogue():
                        osb = ep.tile([65, 512], F32, tag="osb")
                        if last:
                            # final chunk: copy+DMA in two halves on DVE so
                            # the first half's DMA overlaps the second copy
                            nc.vector.tensor_copy(osb[:, 0:256], acc[:, 0:256])
                            nc.sync.dma_start(
                                out=o_d[:, 512 * m : 512 * m + 256],
                                in_=osb[:, 0:256],
                            )
                            nc.vector.tensor_copy(
                                osb[:, 256:512], acc[:, 256:512]
                            )
                            dve_t[0] += 800.0
                        else:
                            # split the PSUM->SBUF copy across DVE and ACT
                            nc.vector.tensor_copy(osb[:, 0:256], acc[:, 0:256])
                            nc.scalar.activation(
                                osb[:, 256:512],
                                acc[:, 256:512],
                                mybir.ActivationFunctionType.Copy,
                            )
                            act_t[0] += 360.0
                            dve_t[0] += 360.0
                        if last:
                            nc.sync.dma_start(
                                out=o_d[:, 512 * m + 256 : 512 * (m + 1)],
                                in_=osb[:, 256:512],
                            )
                        else:
                            nc.sync.dma_start(
                                out=o_d[:, 512 * m : 512 * (m + 1)], in_=osb[:]
                            )

                    return epilogue

                n_gangs = (C + 1) // 2
                positions = list(range(0, C, 2))
                if not short_rows and C >= 12:
                    # spread the two diag gangs (forced-DVE stt) between DR
                    # gangs so ACT isn't starved at chunk tails
                    if diag_first:
                        i = positions.index(ns)
                        positions[i + 1], positions[i + 2] = (
                            positions[i + 2], positions[i + 1],
                        )
                    else:
                        positions[-3], positions[-2] = (
                            positions[-2], positions[-3],
                        )
                for gi, t0 in enumerate(positions):
                    gang = (t0, t0 + 1)
                    sc = ps_sc.tile([128, 2, 512], F32, tag="sc")
                    geoms = []
                    for j, t in enumerate(gang):
                        g, off, w = tile_geom(t)
                        geoms.append((g, off, w))
                        if t < ns:
                            blk = t // 2
                            lhsT = ktm[
                                64 * j : 64 * (j + 1), 128 * blk : 128 * (blk + 1)
                            ]
                        else:
                            p = t - ns
                            blk = p // 2
                            lhsT = kts[
                                64 * j : 64 * (j + 1),
                                512 * m + 128 * blk : 512 * m + 128 * (blk + 1),
                            ]
                        rhs = qt[64 * j : 64 * (j + 1), q_sl]
                        if off:
                            rhs = rhs[:, off:512]
                        # diag scores land at their true q-columns so the
                        # full-width DoubleRow PV accumulates them unshifted;
                        # the short-row fp16 path keeps the 0-based window
                        dst = (
                            sc[:, j, 0:w] if short_rows else sc[:, j, off:512]
                        )
                        nc.tensor.matmul(
                            dst,
                            lhsT,
                            rhs,
                            start=True,
                            stop=True,
                        )
                    pt = pp.tile([128, 2, 512], F16, tag="pt")
                    is_diag = any(g >= 0 for g, _, _ in geoms)
                    if short_rows:
                        # fp16 path, mask applied via tensor_mul later.
                        # Even tiles exact on ACT (covers the shortest rows);
                        # odd tiles Schraudolph-fp16 on DVE (rows there have
                        # >=129 keys, enough wash-out).
                        pti = pt.bitcast(I16)
                        for j, (g, off, w) in enumerate(geoms):
                            if gang[j] % 2 == 0:
                                nc.scalar.activation(
                                    pt[:, j, 0:w], sc[:, j, 0:w], EXP,
                                    scale=0.125, bias=biasC[:],
                                )
                                act_t[0] += 0.87 * w + 90.0
                            else:
                                nc.vector.tensor_scalar(
                                    pti[:, j, 0:w], sc[:, j, 0:w],
                                    SCH_A, SCH_B, MUL, ADD,
                                )
                                dve_t[0] += 1.0417 * w + 70.0
                        dve_t[0] += 0.52 * sum(w for _, _, w in geoms) + 300
                    elif is_diag:
                        # fused exp+mask Schraudolph -> fp8 on DVE, narrowed
                        # to the valid window; GPSIMD zeroes the stale cols
                        ptu8 = pt.bitcast(U8)
                        v = 0 if geoms[0][0] == 0 else 1  # mb01 or mb23
                        for j, (g, off, w) in enumerate(geoms):
                            if off:
                                nc.gpsimd.memset(ptu8[:, j, 0:off], 0)
                            mbs = mb[:, j, 512 * v + off : 512 * (v + 1)]
                            plan_exp(w, allow=("dve",))
                            nc.vector.scalar_tensor_tensor(
                                ptu8[:, j, off:512], sc[:, j, off:512],
                                SCH_A8, mbs, MUL, ADD,
                            )
                    else:
                        pt8 = pt.bitcast(F8)
                        ptu8 = pt.bitcast(U8)
                        if dr_no[0] < 3:
                            # pipeline fill: split first gangs across engines
                            nc.scalar.activation(
                                pt8[:, 0, 0:512], sc[:, 0, :], EXP,
                                scale=0.125, bias=biasC[:],
                            )
                            nc.vector.tensor_scalar(
                                ptu8[:, 1, 0:512], sc[:, 1, :],
                                SCH_A8, SCH_B8, MUL, ADD,
                            )
                            act_t[0] += 0.87 * 512 + 90.0
                            dve_t[0] += 1.0417 * 512 + 70.0
                        else:
                            # tail: keep DVE free for the final copy chain
                            force = (
                                ("act",)
                                if (m == 3 and gi >= n_gangs - 2)
                                else ("act", "dve")
                            )
                            eng = plan_exp(1024, allow=force)
                            if eng == "act":
                                nc.scalar.activation(
                                    pt8[:, :, 0:512], sc[:], EXP,
                                    scale=0.125, bias=biasC[:],
                                )
                            else:
                                nc.vector.tensor_scalar(
                                    ptu8[:, :, 0:512], sc[:],
                                    SCH_A8, SCH_B8, MUL, ADD,
                                )
                    after = make_epilogue() if gi == n_gangs - 1 else None
                    pending.append((emit_pv, pt, gang, after))
                    if not is_diag and not short_rows:
                        dr_no[0] += 1
                    gang_no[0] += 1
                    # plugs must stop before the first start=False PV pop
                    # (they'd clobber live accumulation in the acc bufs)
                    if gang_no[0] <= 3:
                        # fill plugs: dependency-free PE work in the acc bank
                        # keeps the PE dense (HAM-warm) while the first
                        # gangs' exp results are still in flight (the next
                        # real PV's start=True clears the plug garbage)
                        for _ in range(2 + (gang_no[0] <= 2)):
                            wpa = ps_acc.tile([65, 512], F32, tag="acc")
                            nc.tensor.matmul(
                                wpa[:], warm[:, 0:65], warm[:],
                                start=True, stop=True,
                            )
                    pump(2 if m == 0 else 3)
            pump(0)

    nc.compile()
    return nc


def _prep_core_inputs(slot_c, b, query, key, value):
    n_shared = [max(c - 8, 0) for c in slot_c]
    n_slab = [min(c, 8) for c in slot_c]
    nb_shared = [ns // 2 for ns in n_shared]
    n_pr = [c // 2 for c in slot_c]
    max_nb = max(nb_shared)
    max_nb8 = max(n_pr)
    W8 = 80 * max_nb8

    qt = np.zeros((128, 2048), np.float16)
    kts = np.zeros((128, 2048), np.float16)
    vs = np.zeros((128, 260), np.float16)
    # ktm: block j holds tiles 2j (rows 0-63) and 2j+1 (rows 64-127)
    ktm = np.zeros((128, 128 * max(max_nb, 1)), np.float16)
    kT = key[b].T.astype(np.float16)  # [64, S]
    for j in range(max_nb):
        ktm[0:64, 128 * j : 128 * (j + 1)] = kT[:, 128 * (2 * j) : 128 * (2 * j + 1)]
        ktm[64:128, 128 * j : 128 * (j + 1)] = kT[
            :, 128 * (2 * j + 1) : 128 * (2 * j + 2)
        ]
    vaug = np.ones((S, 65), np.float16)
    vaug[:, :64] = value[b]
    # vm8: V pairs (all tiles), fp8e4m3, padded to stride 80
    vm8 = np.zeros((128, 2, W8), E4NP)
    v8 = vaug.astype(E4NP)  # RNE quantization
    for p in range(max_nb8):
        for j in range(2):
            t = 2 * p + j
            vm8[:, j, 80 * p : 80 * p + 65] = v8[128 * t : 128 * (t + 1), :]
    for m in range(4):
        c = _chunk_index(slot_c, m)
        n = slot_c[m]
        diag_first = m == 3 and n_shared[m] >= 4
        qchunk = query[b, 512 * c : 512 * (c + 1), :].T.astype(np.float16)
        qt[0:64, 512 * m : 512 * (m + 1)] = qchunk
        qt[64:128, 512 * m : 512 * (m + 1)] = qchunk
        for p in range(n_slab[m]):
            if diag_first:
                t = (n - 4 + p) if p < 4 else (n - 8 + (p - 4))
            else:
                t = n_shared[m] + p
            row = slice(0, 64) if p % 2 == 0 else slice(64, 128)
            col = slice(512 * m + 128 * (p // 2), 512 * m + 128 * (p // 2 + 1))
            kts[row, col] = key[b, 128 * t : 128 * (t + 1), :].T
        if n == 4:  # short-row chunk: fp16 V for tiles 0..3
            for g in range(4):
                vs[:, 65 * g : 65 * (g + 1)] = vaug[128 * g : 128 * (g + 1), :]
    mask = np.triu(np.ones((128, 512), dtype=np.float16))
    # mb: fused Schraudolph-bias masks for diag gangs, [128, 2, 1024]
    # variant v=0 (tiles g=0,1), v=1 (tiles g=2,3); scores sit at their true
    # q-columns, so cols < off are stale -> MASKB, the [off, off+128) block
    # is the causal triangle, and cols >= off+128 are fully allowed
    mb = np.full((128, 2, 1024), MASKB, np.float16)
    tri128 = np.triu(np.ones((128, 128), bool))
    for v in range(2):
        for j in range(2):
            off = 128 * (2 * v + j)
            blk = np.full((128, 512), MASKB, np.float16)
            blk[:, off : off + 128] = np.where(
                tri128, np.float16(SCH_B8), np.float16(MASKB)
            )
            blk[:, off + 128 :] = np.float16(SCH_B8)
            mb[:, j, 512 * v : 512 * (v + 1)] = blk
    return {
        "qt": qt, "ktm": ktm, "kts": kts, "vm8": vm8, "vs": vs,
        "mask": mask, "mb": mb,
    }


def _make_runner(nc, devices):
    """Vendored multi-core run_bass_via_pjrt with an explicit device set,
    split into an async dispatch and a blocking unpack."""
    from jax.sharding import Mesh, PartitionSpec

    bass2jax.install_neuronx_cc_hook()
    n = len(devices)
    partition_name = nc.partition_id_tensor.name if nc.partition_id_tensor else None
    in_names, out_names, out_avals, zero_outs = [], [], [], []
    for alloc in nc.m.functions[0].allocations:
        if not isinstance(alloc, mybir.MemoryLocationSet):
            continue
        name = alloc.memorylocations[0].name
        if alloc.kind == "ExternalInput":
            if name != partition_name:
                in_names.append(name)
        elif alloc.kind == "ExternalOutput":
            out_names.append(name)
            shape = tuple(alloc.tensor_shape)
            dtype = mybir.dt.np(alloc.dtype)
            out_avals.append(jax.core.ShapedArray(shape, dtype))
            zero_outs.append(np.zeros(shape, dtype))
    n_params = len(in_names)
    all_in = list(in_names) + list(out_names)
    if partition_name is not None:
        all_in.append(partition_name)
    all_in = tuple(all_in)
    donate = tuple(range(n_params, n_params + len(out_names)))

    def _body(*args):
        operands = list(args)
        if partition_name is not None:
            operands.append(bass2jax.partition_id_tensor())
        outs = bass2jax._bass_exec_p.bind(
            *operands,
            out_avals=tuple(out_avals),
            in_names=all_in,
            out_names=tuple(out_names),
            lowering_input_output_aliases=(),
            sim_require_finite=True,
            sim_require_nnan=True,
            nc=nc,
        )
        return tuple(outs)

    mesh = Mesh(np.asarray(devices), ("core",))
    in_specs = (PartitionSpec("core"),) * (n_params + len(out_names))
    out_specs = (PartitionSpec("core"),) * len(out_names)
    sharded = jax.jit(
        jax.shard_map(
            _body, mesh=mesh, in_specs=in_specs, out_specs=out_specs, check_vma=False
        ),
        donate_argnums=donate,
        keep_unused=True,
    )

    def dispatch(in_maps):
        concat_in = [
            np.concatenate([np.asarray(in_maps[c][nm]) for c in range(n)], axis=0)
            for nm in in_names
        ]
        concat_zeros = [
            np.zeros((n * z.shape[0], *z.shape[1:]), z.dtype) for z in zero_outs
        ]
        return sharded(*concat_in, *concat_zeros)

    def unpack(out_arrs):
        return [
            {
                nm: np.asarray(out_arrs[i]).reshape(n, *out_avals[i].shape)[c]
                for i, nm in enumerate(out_names)
            }
            for c in range(n)
        ]

    return dispatch, unpack


def _get_engine():
    if "engine" not in _cache:
        devs = jax.devices()
        ncA = _build_program(SLOT_A, 10, act_seed=0.0)
        ncB = _build_program(SLOT_B, 7, act_seed=0.0, split_qt=True)
        dispA, unpackA = _make_runner(ncA, devs[0:4])
        dispB, unpackB = _make_runner(ncB, devs[4:8])
        _cache["engine"] = (dispA, unpackA, dispB, unpackB)
        _cache["ncs"] = (ncA, ncB)
    return _cache["engine"]


def run(query, key, value):
    dispA, unpackA, dispB, unpackB = _get_engine()
    mapsA = [_prep_core_inputs(SLOT_A, b, query, key, value) for b in range(4)]
    mapsB = [_prep_core_inputs(SLOT_B, b, query, key, value) for b in range(4)]
    outA = dispA(mapsA)
    outB = dispB(mapsB)
    resA = unpackA(outA)
    resB = unpackB(outB)

    out = np.zeros((B, S, D), np.float32)
    for b in range(4):
        for slot_c, res in ((SLOT_A, resA[b]), (SLOT_B, resB[b])):
            o = res["o"]  # [65, 2048]: chunk m at cols [512m, 512m+512)
            for m in range(4):
                c = _chunk_index(slot_c, m)
                blk = o[:, 512 * m : 512 * (m + 1)]
                out[b, 512 * c : 512 * (c + 1), :] = (blk[:64] / blk[64]).T
    return out


def kernel(query, key, value):
    query = np.ascontiguousarray(np.asarray(query, dtype=np.float32))
    key = np.ascontiguousarray(np.asarray(key, dtype=np.float32))
    value = np.ascontiguousarray(np.asarray(value, dtype=np.float32))
    return run(query, key, value)
